# revision 1
# baseline (speedup 1.0000x reference)
"""DFN Bokeh model on 8 TRN2 NeuronCores.

Sharding: 8 shards = (batch b, H-half) pairs; each core gets a 278-row slab
(256 out rows + 11-row conv halo each side, zero-padded at image edges by the
host). Conv chain (5x conv3x3+relu, then the 2 softmax-logit convs fused) runs
as f32r matmuls with channels on partitions, PSUM row accumulation, ping-pong
DRAM slabs. The separable per-pixel filter runs with image rows on partitions;
the horizontal-pass output is bounced through a DRAM "plane" with replicate
rows so the vertical taps become plain row-offset DMA loads.
"""

import numpy as np
import sys

sys.path.insert(0, "/opt/trn_rl_repo")

B, H, W = 4, 512, 512
K = 11
RS = 256          # out rows per core
HS = RS + 22      # slab rows (conv halo 11 each side)
NBUF = HS + 2     # slab buffer rows incl. zero conv pad
WP = W + 2        # slab cols incl. zero conv pad
RB = 8            # conv rows per block (PSUM banks)
F32 = None        # set after import

_prog = None


def _build_program():
    import concourse.bacc as bacc
    import concourse.mybir as mybir
    from concourse import tile

    f32 = mybir.dt.float32
    f32r = mybir.dt.float32r
    AF = mybir.ActivationFunctionType

    nc = bacc.Bacc("TRN2", target_bir_lowering=False, debug=False, num_devices=8)

    x = nc.dram_tensor("x", [4, NBUF, WP], f32, kind="ExternalInput").ap()
    rgbf = nc.dram_tensor("rgbf", [3, HS, W + 10], f32, kind="ExternalInput").ap()
    masks = nc.dram_tensor("masks", [4, 5], f32, kind="ExternalInput").ap()
    wnames = {
        "wl1": (12, 192), "wl2a": (128, 192), "wl2b": (64, 192),
        "wl3a": (128, 384), "wl3b": (64, 384), "wl4": (128, 1152),
        "wl5": (128, 576), "wl6a": (128, 66), "wl6b": (64, 66),
    }
    wd = {n: nc.dram_tensor(n, list(s), f32, kind="ExternalInput").ap()
          for n, s in wnames.items()}
    bshapes = {"bl1": 64, "bl2": 64, "bl3": 128, "bl4": 128, "bl5": 64, "bl6": 22}
    bd = {n: nc.dram_tensor(n, [c, 1], f32, kind="ExternalInput").ap()
          for n, c in bshapes.items()}
    out = nc.dram_tensor("out", [3, RS, W], f32, kind="ExternalOutput").ap()

    with tile.TileContext(nc) as tc:
        with (
            tc.tile_pool(name="dram", bufs=1, space="DRAM") as dpool,
            tc.tile_pool(name="wts", bufs=1) as wpool,
        ):
            A = dpool.tile([128, NBUF, WP], f32, tag="A")
            Bd = dpool.tile([128, NBUF, WP], f32, tag="B")
            C = dpool.tile([22, NBUF, WP], f32, tag="C")
            P = dpool.tile([3, HS - 12 + 10, W], f32, tag="P")  # 266 rows

            wt = {}
            for n, s in wnames.items():
                t = wpool.tile(list(s), f32r, tag=n)
                nc.sync.dma_start(out=t[:], in_=wd[n][:].bitcast(f32r))
                wt[n] = t
            bt = {}
            for n, c in bshapes.items():
                t = wpool.tile([c, 1], f32, tag=n)
                nc.sync.dma_start(out=t[:], in_=bd[n][:])
                bt[n] = t
            # mk cols: 0=keep_top 1=rep_top (partitions 0-4),
            #          2=keep_bot 3=rep_bot (partitions 5-9)
            mk = wpool.tile([16, 4], f32, tag="mk")
            nc.vector.memset(mk[0:5, 2:3], 1.0)
            nc.vector.memset(mk[0:5, 3:4], 0.0)
            nc.sync.dma_start(out=mk[0:5, 0:1], in_=masks[0:1, :])
            nc.sync.dma_start(out=mk[0:5, 1:2], in_=masks[1:2, :])
            nc.sync.dma_start(out=mk[5:10, 2:3], in_=masks[2:3, :])
            nc.sync.dma_start(out=mk[5:10, 3:4], in_=masks[3:4, :])

            cb = wpool.tile([128, 1], f32, tag="cb")
            nc.vector.memset(cb[:], -3.0)

            # zero the pad rows of A and B
            with tc.tile_pool(name="zz", bufs=1) as zpool:
                z = zpool.tile([128, WP], f32, tag="z")
                nc.vector.memset(z[:], 0.0)
                for dst in (A, Bd):
                    nc.sync.dma_start(out=dst[:, 0, :], in_=z[:])
                    nc.sync.dma_start(out=dst[:, NBUF - 1, :], in_=z[:])

            # ---------------- conv chain ----------------
            def conv_layer(src, dst, mode, cin, cout, wkey, bkey, func, cpool,
                           opool, ppool):
                blocks = [(r0, min(RB, HS - r0)) for r0 in range(0, HS, RB)]
                for r0, rb in blocks:
                    it = cpool.tile([128, RB + 2, WP], f32r, tag="cin")
                    if mode == "p3":
                        nc.sync.dma_start(out=it[0:4, 0:rb + 2, :],
                                          in_=src[0:4, r0:r0 + rb + 2, :].bitcast(f32r))
                        nc.sync.dma_start(out=it[4:8, 0:rb, :],
                                          in_=src[0:4, r0 + 1:r0 + rb + 1, :].bitcast(f32r))
                        nc.sync.dma_start(out=it[8:12, 0:rb, :],
                                          in_=src[0:4, r0 + 2:r0 + rb + 2, :].bitcast(f32r))
                        groups = [(lambda dx: wt[wkey][:, dx * cout:(dx + 1) * cout],
                                   lambda rr, dx: it[0:12, rr, dx:dx + W])]
                    elif mode == "p2":
                        nc.sync.dma_start(out=it[0:64, 0:rb + 2, :],
                                          in_=src[0:64, r0:r0 + rb + 2, :].bitcast(f32r))
                        nc.sync.dma_start(out=it[64:128, 0:rb, :],
                                          in_=src[0:64, r0 + 1:r0 + rb + 1, :].bitcast(f32r))
                        ka, kb = wkey
                        groups = [
                            (lambda dx: wt[ka][:, dx * cout:(dx + 1) * cout],
                             lambda rr, dx: it[0:128, rr, dx:dx + W]),
                            (lambda dx: wt[kb][:, dx * cout:(dx + 1) * cout],
                             lambda rr, dx: it[0:64, rr + 2, dx:dx + W]),
                        ]
                    else:  # f3
                        nc.sync.dma_start(out=it[0:128, 0:rb + 2, :],
                                          in_=src[0:128, r0:r0 + rb + 2, :].bitcast(f32r))
                        groups = [
                            (lambda dx, dy=dy: wt[wkey][:, (dy * 3 + dx) * cout:
                                                        (dy * 3 + dx + 1) * cout],
                             lambda rr, dx, dy=dy: it[0:128, rr + dy, dx:dx + W])
                            for dy in range(3)
                        ]
                    pp = [ppool.tile([cout, W], f32, tag="pp", name="pp%d" % i)
                          for i in range(rb)]
                    ng = len(groups)
                    for gi, (sf, mf) in enumerate(groups):
                        for dx in range(3):
                            stat = sf(dx)
                            for rr in range(rb):
                                nc.tensor.matmul(
                                    pp[rr][:], lhsT=stat,
                                    rhs=mf(rr, dx),
                                    start=(gi == 0 and dx == 0),
                                    stop=(gi == ng - 1 and dx == 2),
                                )
                    ot = opool.tile([128, RB, WP], f32, tag="cout")
                    nc.vector.memset(ot[0:cout, 0:rb, 0:1], 0.0)
                    nc.vector.memset(ot[0:cout, 0:rb, WP - 1:WP], 0.0)
                    for rr in range(rb):
                        if func is None:
                            nc.any.tensor_scalar(
                                ot[0:cout, rr, 1:1 + W], pp[rr][:],
                                bt[bkey][:], 0.0,
                                mybir.AluOpType.add, mybir.AluOpType.max)
                        else:
                            nc.scalar.activation(ot[0:cout, rr, 1:1 + W],
                                                 pp[rr][:], func,
                                                 bias=bt[bkey][:])
                    nc.sync.dma_start(out=dst[0:cout, r0 + 1:r0 + 1 + rb, :],
                                      in_=ot[0:cout, 0:rb, :])

            with (
                tc.tile_pool(name="cin", bufs=3) as cpool,
                tc.tile_pool(name="cout", bufs=3) as opool,
                tc.tile_pool(name="psum", bufs=8, space="PSUM") as ppool,
            ):
                LRelu, LExp = None, AF.Exp
                conv_layer(x, A, "p3", 4, 64, "wl1", "bl1", LRelu, cpool, opool, ppool)
                conv_layer(A, Bd, "p2", 64, 64, ("wl2a", "wl2b"), "bl2", LRelu,
                           cpool, opool, ppool)
                conv_layer(Bd, A, "p2", 64, 128, ("wl3a", "wl3b"), "bl3", LRelu,
                           cpool, opool, ppool)
                conv_layer(A, Bd, "f3", 128, 128, "wl4", "bl4", LRelu,
                           cpool, opool, ppool)
                conv_layer(Bd, A, "f3", 128, 64, "wl5", "bl5", LRelu,
                           cpool, opool, ppool)
                conv_layer(A, C, "p2", 64, 22, ("wl6a", "wl6b"), "bl6", LExp,
                           cpool, opool, ppool)

            # ---------------- horizontal pass ----------------
            with tc.tile_pool(name="filt", bufs=2) as fp:
                tts = [(6, 128, 0), (134, 128, 1), (262, 10, 2)]
                for t0, nt, ti in tts:
                    ex = fp.tile([128, K, W], f32, tag="exh")
                    for k in range(K):
                        nc.sync.dma_start(out=ex[0:nt, k, :],
                                          in_=C[k, t0 + 1:t0 + 1 + nt, 1:1 + W])
                    sh = fp.tile([128, W], f32, tag="sh")
                    nc.vector.tensor_add(sh[0:nt], ex[0:nt, 0, :], ex[0:nt, 1, :])
                    for k in range(2, K):
                        nc.vector.tensor_add(sh[0:nt], sh[0:nt], ex[0:nt, k, :])
                    rcp = fp.tile([128, W], f32, tag="rcp")
                    nc.vector.reciprocal(rcp[0:nt], sh[0:nt])
                    for c in range(3):
                        rg = fp.tile([128, W + 10], f32, tag="rg")
                        nc.sync.dma_start(out=rg[0:nt, :],
                                          in_=rgbf[c, t0:t0 + nt, :])
                        acc = fp.tile([128, W], f32, tag="acc")
                        tmp = fp.tile([128, W], f32, tag="tmp")
                        nc.vector.tensor_mul(acc[0:nt], ex[0:nt, 0, :],
                                             rg[0:nt, 0:W])
                        for k in range(1, K):
                            nc.vector.tensor_mul(tmp[0:nt], ex[0:nt, k, :],
                                                 rg[0:nt, k:k + W])
                            nc.vector.tensor_add(acc[0:nt], acc[0:nt], tmp[0:nt])
                        oh = fp.tile([128, W], f32, tag="oh")
                        nc.vector.tensor_mul(oh[0:nt], acc[0:nt], rcp[0:nt])
                        if ti == 0:
                            bc = fp.tile([16, W], f32, tag="bc")
                            for i in range(5):
                                nc.sync.dma_start(out=bc[i:i + 1, :],
                                                  in_=oh[5:6, :])
                            nc.vector.tensor_scalar_mul(
                                oh[0:5], oh[0:5], mk[0:5, 0:1])
                            nc.vector.tensor_scalar_mul(
                                bc[0:5], bc[0:5], mk[0:5, 1:2])
                            nc.vector.tensor_add(oh[0:5], oh[0:5], bc[0:5])
                        if ti == 2:
                            bc = fp.tile([16, W], f32, tag="bc")
                            nc.vector.memset(bc[0:5, :], 0.0)
                            for i in range(5):
                                nc.sync.dma_start(out=bc[5 + i:6 + i, :],
                                                  in_=oh[4:5, :])
                            nc.vector.tensor_scalar_mul(
                                oh[0:10], oh[0:10], mk[0:10, 2:3])
                            nc.vector.tensor_scalar_mul(
                                bc[0:10], bc[0:10], mk[0:10, 3:4])
                            nc.vector.tensor_add(oh[0:10], oh[0:10], bc[0:10])
                        nc.sync.dma_start(out=P[c, t0 - 6:t0 - 6 + nt, :],
                                          in_=oh[0:nt, :])

            # ---------------- vertical pass + blend ----------------
            with tc.tile_pool(name="vert", bufs=2) as vp:
                for r0 in (0, 128):
                    ev = vp.tile([128, K, W], f32, tag="ev")
                    for k in range(K):
                        nc.sync.dma_start(out=ev[:, k, :],
                                          in_=C[K + k, r0 + 12:r0 + 140, 1:1 + W])
                    sv = vp.tile([128, W], f32, tag="sv")
                    nc.vector.tensor_add(sv[:], ev[:, 0, :], ev[:, 1, :])
                    for k in range(2, K):
                        nc.vector.tensor_add(sv[:], sv[:], ev[:, k, :])
                    rcpv = vp.tile([128, W], f32, tag="rcpv")
                    nc.vector.reciprocal(rcpv[:], sv[:])
                    dep = vp.tile([128, W], f32, tag="dep")
                    nc.sync.dma_start(out=dep[:], in_=x[3, r0 + 12:r0 + 140, 1:1 + W])
                    msk = vp.tile([128, W], f32, tag="msk")
                    nc.scalar.activation(msk[:], dep[:], AF.Sigmoid,
                                         bias=cb[:], scale=15.0)
                    for c in range(3):
                        pt = vp.tile([128, K, W], f32, tag="pt")
                        for k in range(K):
                            nc.sync.dma_start(out=pt[:, k, :],
                                              in_=P[c, r0 + k:r0 + k + 128, :])
                        accv = vp.tile([128, W], f32, tag="accv")
                        tmpv = vp.tile([128, W], f32, tag="tmpv")
                        nc.vector.tensor_mul(accv[:], ev[:, 0, :], pt[:, 0, :])
                        for k in range(1, K):
                            nc.vector.tensor_mul(tmpv[:], ev[:, k, :], pt[:, k, :])
                            nc.vector.tensor_add(accv[:], accv[:], tmpv[:])
                        blur = vp.tile([128, W], f32, tag="blur")
                        nc.vector.tensor_mul(blur[:], accv[:], rcpv[:])
                        rgt = vp.tile([128, W], f32, tag="rgt")
                        nc.sync.dma_start(out=rgt[:],
                                          in_=x[c, r0 + 12:r0 + 140, 1:1 + W])
                        dt_ = vp.tile([128, W], f32, tag="dt")
                        nc.vector.tensor_sub(dt_[:], rgt[:], blur[:])
                        nc.vector.tensor_mul(dt_[:], dt_[:], msk[:])
                        ov = vp.tile([128, W], f32, tag="ov")
                        nc.vector.tensor_add(ov[:], blur[:], dt_[:])
                        nc.sync.dma_start(out=out[c, r0:r0 + 128, :], in_=ov[:])

    nc.compile()
    return nc


def _pack_weights(ins):
    f = np.float32
    w1, w2, w3, w4, w5 = ins["w1"], ins["w2"], ins["w3"], ins["w4"], ins["w5"]
    wcat = np.concatenate([ins["wh"], ins["wv"]], axis=0)  # [22,64,3,3]

    def packp3(w):  # [64,4,3,3] -> [12, 3*64]
        o = np.zeros((12, 3 * w.shape[0]), f)
        for dy in range(3):
            for dx in range(3):
                o[dy * 4:(dy + 1) * 4, dx * w.shape[0]:(dx + 1) * w.shape[0]] = \
                    w[:, :, dy, dx].T
        return o

    def packp2(w):  # [O,64,3,3] -> ([128,3O],[64,3O])
        O = w.shape[0]
        a = np.zeros((128, 3 * O), f)
        b = np.zeros((64, 3 * O), f)
        for dx in range(3):
            a[0:64, dx * O:(dx + 1) * O] = w[:, :, 0, dx].T
            a[64:128, dx * O:(dx + 1) * O] = w[:, :, 1, dx].T
            b[:, dx * O:(dx + 1) * O] = w[:, :, 2, dx].T
        return a, b

    def packf3(w):  # [O,128,3,3] -> [128, 9O]
        O = w.shape[0]
        a = np.zeros((128, 9 * O), f)
        for dy in range(3):
            for dx in range(3):
                a[:, (dy * 3 + dx) * O:(dy * 3 + dx + 1) * O] = w[:, :, dy, dx].T
        return a

    w2a, w2b = packp2(w2)
    w3a, w3b = packp2(w3)
    w6a, w6b = packp2(wcat)
    d = {"wl1": packp3(w1), "wl2a": w2a, "wl2b": w2b, "wl3a": w3a, "wl3b": w3b,
         "wl4": packf3(w4), "wl5": packf3(w5), "wl6a": w6a, "wl6b": w6b}
    for i, n in enumerate(["bl1", "bl2", "bl3", "bl4", "bl5"]):
        d[n] = np.ascontiguousarray(ins["b%d" % (i + 1)][:, None], f)
    d["bl6"] = np.ascontiguousarray(
        np.concatenate([ins["bh"], ins["bv"]])[:, None], f)
    return d


def kernel(**inputs):
    global _prog
    from concourse.bass_utils import run_bass_kernel_spmd

    if _prog is None:
        _prog = _build_program()
    nc = _prog

    rgb = np.asarray(inputs["rgb"], np.float32)
    depth = np.asarray(inputs["depth"], np.float32)
    wd = _pack_weights({k: np.asarray(v, np.float32) for k, v in inputs.items()
                        if k not in ("rgb", "depth")})

    x_full = np.concatenate([rgb, depth], axis=1)  # [B,4,H,W]
    rgb_pad = np.pad(rgb, ((0, 0), (0, 0), (11, 11), (5, 5)), mode="edge")

    in_maps = []
    shards = []
    for core in range(8):
        b, half = core // 2, core % 2
        s = half * RS
        xs = np.zeros((4, NBUF, WP), np.float32)
        lo, hi = s - 11, s + RS + 11
        clo, chi = max(lo, 0), min(hi, H)
        xs[:, 1 + (clo - lo):1 + (chi - lo), 1:1 + W] = x_full[b, :, clo:chi, :]
        rf = np.ascontiguousarray(rgb_pad[b, :, s:s + HS, :])
        keep_top = 0.0 if half == 0 else 1.0
        keep_bot = 0.0 if half == 1 else 1.0
        mrows = np.zeros((4, 5), np.float32)
        mrows[0, :] = keep_top
        mrows[1, :] = 1.0 - keep_top
        mrows[2, :] = keep_bot
        mrows[3, :] = 1.0 - keep_bot
        m = {"x": xs, "rgbf": rf, "masks": mrows}
        m.update(wd)
        in_maps.append(m)
        shards.append((b, s))

    res = run_bass_kernel_spmd(nc, in_maps, core_ids=list(range(8)))
    outp = np.zeros((B, 3, H, W), np.float32)
    for core, (b, s) in enumerate(shards):
        outp[b, :, s:s + RS, :] = res.results[core]["out"]
    return outp



# revision 4
# speedup vs baseline: 560.0267x; 560.0267x over previous
"""DFN Bokeh model on 8 TRN2 NeuronCores.

Sharding: 8 shards = (batch b, H-half) pairs; each core gets a 278-row slab
(256 out rows + 11-row conv halo each side, zero-padded at image edges by the
host). Conv chain (5x conv3x3+relu, then the 2 softmax-logit convs fused) runs
as f32r matmuls with channels on partitions, PSUM row accumulation, ping-pong
DRAM slabs. The separable per-pixel filter runs with image rows on partitions;
the horizontal-pass output is bounced through a DRAM "plane" with replicate
rows so the vertical taps become plain row-offset DMA loads.
"""

import numpy as np
import sys

sys.path.insert(0, "/opt/trn_rl_repo")

B, H, W = 4, 512, 512
K = 11
RS = 256          # out rows per core
HS = RS + 22      # slab rows (conv halo 11 each side)
NBUF = HS + 2     # slab buffer rows incl. zero conv pad
WP = W + 2        # slab cols incl. zero conv pad
RB = 8            # conv rows per block (PSUM banks)
F32 = None        # set after import

_prog = None


def _build_program():
    import concourse.bacc as bacc
    import concourse.mybir as mybir
    from concourse import tile

    f32 = mybir.dt.float32
    f32r = mybir.dt.float32r
    AF = mybir.ActivationFunctionType

    nc = bacc.Bacc("TRN2", target_bir_lowering=False, debug=False, num_devices=8)

    x = nc.dram_tensor("x", [4, NBUF, WP], f32, kind="ExternalInput").ap()
    rgbf = nc.dram_tensor("rgbf", [3, HS, W + 10], f32, kind="ExternalInput").ap()
    masks = nc.dram_tensor("masks", [4, 5], f32, kind="ExternalInput").ap()
    wnames = {
        "wl1": (12, 192), "wl2a": (128, 192), "wl2b": (64, 192),
        "wl3a": (128, 384), "wl3b": (64, 384), "wl4": (128, 1152),
        "wl5": (128, 576), "wl6a": (128, 66), "wl6b": (64, 66),
    }
    wd = {n: nc.dram_tensor(n, list(s), f32, kind="ExternalInput").ap()
          for n, s in wnames.items()}
    bshapes = {"bl1": 64, "bl2": 64, "bl3": 128, "bl4": 128, "bl5": 64, "bl6": 22}
    bd = {n: nc.dram_tensor(n, [c, 1], f32, kind="ExternalInput").ap()
          for n, c in bshapes.items()}
    out = nc.dram_tensor("out", [3, RS, W], f32, kind="ExternalOutput").ap()

    with tile.TileContext(nc) as tc:
        with (
            tc.tile_pool(name="dram", bufs=1, space="DRAM") as dpool,
            tc.tile_pool(name="wts", bufs=1) as wpool,
        ):
            A = dpool.tile([128, NBUF, WP], f32, tag="A")
            Bd = dpool.tile([128, NBUF, WP], f32, tag="B")
            C = dpool.tile([22, NBUF, WP], f32, tag="C")
            P = dpool.tile([3, HS - 12 + 10, W], f32, tag="P")  # 266 rows

            wt = {}
            for n, s in wnames.items():
                t = wpool.tile(list(s), f32r, tag=n)
                nc.sync.dma_start(out=t[:], in_=wd[n][:].bitcast(f32r))
                wt[n] = t
            bt = {}
            for n, c in bshapes.items():
                t = wpool.tile([c, 1], f32, tag=n)
                nc.sync.dma_start(out=t[:], in_=bd[n][:])
                bt[n] = t
            # mk cols: 0=keep_top 1=rep_top (partitions 0-4),
            #          2=keep_bot 3=rep_bot (partitions 5-9)
            mk = wpool.tile([16, 4], f32, tag="mk")
            nc.vector.memset(mk[0:5, 2:3], 1.0)
            nc.vector.memset(mk[0:5, 3:4], 0.0)
            nc.sync.dma_start(out=mk[0:5, 0:1], in_=masks[0:1, :])
            nc.sync.dma_start(out=mk[0:5, 1:2], in_=masks[1:2, :])
            nc.sync.dma_start(out=mk[5:10, 2:3], in_=masks[2:3, :])
            nc.sync.dma_start(out=mk[5:10, 3:4], in_=masks[3:4, :])

            cb = wpool.tile([128, 1], f32, tag="cb")
            nc.vector.memset(cb[:], -3.0)

            # zero the pad rows of A and B
            with tc.tile_pool(name="zz", bufs=1) as zpool:
                z = zpool.tile([128, WP], f32, tag="z")
                nc.vector.memset(z[:], 0.0)
                for dst in (A, Bd):
                    nc.sync.dma_start(out=dst[:, 0, :], in_=z[:])
                    nc.sync.dma_start(out=dst[:, NBUF - 1, :], in_=z[:])

            # ---------------- conv chain ----------------
            def conv_layer(src, dst, mode, cin, cout, wkey, bkey, func, cpool,
                           opool, ppool):
                blocks = [(r0, min(RB, HS - r0)) for r0 in range(0, HS, RB)]
                for r0, rb in blocks:
                    it = cpool.tile([128, RB + 2, WP], f32r, tag="cin")
                    if mode == "p3":
                        nc.sync.dma_start(out=it[0:4, 0:rb + 2, :],
                                          in_=src[0:4, r0:r0 + rb + 2, :].bitcast(f32r))
                        nc.sync.dma_start(out=it[4:8, 0:rb, :],
                                          in_=src[0:4, r0 + 1:r0 + rb + 1, :].bitcast(f32r))
                        nc.sync.dma_start(out=it[8:12, 0:rb, :],
                                          in_=src[0:4, r0 + 2:r0 + rb + 2, :].bitcast(f32r))
                        groups = [(lambda dx: wt[wkey][:, dx * cout:(dx + 1) * cout],
                                   lambda rr, dx: it[0:12, rr, dx:dx + W])]
                    elif mode == "p2":
                        nc.sync.dma_start(out=it[0:64, 0:rb + 2, :],
                                          in_=src[0:64, r0:r0 + rb + 2, :].bitcast(f32r))
                        nc.sync.dma_start(out=it[64:128, 0:rb, :],
                                          in_=src[0:64, r0 + 1:r0 + rb + 1, :].bitcast(f32r))
                        ka, kb = wkey
                        groups = [
                            (lambda dx: wt[ka][:, dx * cout:(dx + 1) * cout],
                             lambda rr, dx: it[0:128, rr, dx:dx + W]),
                            (lambda dx: wt[kb][:, dx * cout:(dx + 1) * cout],
                             lambda rr, dx: it[0:64, rr + 2, dx:dx + W]),
                        ]
                    else:  # f3
                        nc.sync.dma_start(out=it[0:128, 0:rb + 2, :],
                                          in_=src[0:128, r0:r0 + rb + 2, :].bitcast(f32r))
                        groups = [
                            (lambda dx, dy=dy: wt[wkey][:, (dy * 3 + dx) * cout:
                                                        (dy * 3 + dx + 1) * cout],
                             lambda rr, dx, dy=dy: it[0:128, rr + dy, dx:dx + W])
                            for dy in range(3)
                        ]
                    pp = [ppool.tile([cout, W], f32, tag="pp", name="pp%d" % i)
                          for i in range(rb)]
                    ng = len(groups)
                    for gi, (sf, mf) in enumerate(groups):
                        for dx in range(3):
                            stat = sf(dx)
                            for rr in range(rb):
                                nc.tensor.matmul(
                                    pp[rr][:], lhsT=stat,
                                    rhs=mf(rr, dx),
                                    start=(gi == 0 and dx == 0),
                                    stop=(gi == ng - 1 and dx == 2),
                                )
                    ot = opool.tile([128, RB, WP], f32, tag="cout")
                    nc.vector.memset(ot[0:cout, 0:rb, 0:1], 0.0)
                    nc.vector.memset(ot[0:cout, 0:rb, WP - 1:WP], 0.0)
                    for rr in range(rb):
                        if func is None:
                            nc.any.tensor_scalar(
                                ot[0:cout, rr, 1:1 + W], pp[rr][:],
                                bt[bkey][:], 0.0,
                                mybir.AluOpType.add, mybir.AluOpType.max)
                        else:
                            nc.scalar.activation(ot[0:cout, rr, 1:1 + W],
                                                 pp[rr][:], func,
                                                 bias=bt[bkey][:])
                    nc.sync.dma_start(out=dst[0:cout, r0 + 1:r0 + 1 + rb, :],
                                      in_=ot[0:cout, 0:rb, :])

            with (
                tc.tile_pool(name="cin", bufs=3) as cpool,
                tc.tile_pool(name="cout", bufs=3) as opool,
                tc.tile_pool(name="psum", bufs=8, space="PSUM") as ppool,
            ):
                LRelu, LExp = None, AF.Exp
                conv_layer(x, A, "p3", 4, 64, "wl1", "bl1", LRelu, cpool, opool, ppool)
                conv_layer(A, Bd, "p2", 64, 64, ("wl2a", "wl2b"), "bl2", LRelu,
                           cpool, opool, ppool)
                conv_layer(Bd, A, "p2", 64, 128, ("wl3a", "wl3b"), "bl3", LRelu,
                           cpool, opool, ppool)
                conv_layer(A, Bd, "f3", 128, 128, "wl4", "bl4", LRelu,
                           cpool, opool, ppool)
                conv_layer(Bd, A, "f3", 128, 64, "wl5", "bl5", LRelu,
                           cpool, opool, ppool)
                conv_layer(A, C, "p2", 64, 22, ("wl6a", "wl6b"), "bl6", LExp,
                           cpool, opool, ppool)

            # ---------------- horizontal pass ----------------
            with tc.tile_pool(name="filt", bufs=2) as fp:
                tts = [(6, 128, 0), (134, 128, 1), (262, 10, 2)]
                for t0, nt, ti in tts:
                    ex = fp.tile([128, K, W], f32, tag="exh")
                    for k in range(K):
                        nc.sync.dma_start(out=ex[0:nt, k, :],
                                          in_=C[k, t0 + 1:t0 + 1 + nt, 1:1 + W])
                    sh = fp.tile([128, W], f32, tag="sh")
                    nc.vector.tensor_add(sh[0:nt], ex[0:nt, 0, :], ex[0:nt, 1, :])
                    for k in range(2, K):
                        nc.vector.tensor_add(sh[0:nt], sh[0:nt], ex[0:nt, k, :])
                    rcp = fp.tile([128, W], f32, tag="rcp")
                    nc.vector.reciprocal(rcp[0:nt], sh[0:nt])
                    for c in range(3):
                        rg = fp.tile([128, W + 10], f32, tag="rg")
                        nc.sync.dma_start(out=rg[0:nt, :],
                                          in_=rgbf[c, t0:t0 + nt, :])
                        acc = fp.tile([128, W], f32, tag="acc")
                        tmp = fp.tile([128, W], f32, tag="tmp")
                        nc.vector.tensor_mul(acc[0:nt], ex[0:nt, 0, :],
                                             rg[0:nt, 0:W])
                        for k in range(1, K):
                            nc.vector.tensor_mul(tmp[0:nt], ex[0:nt, k, :],
                                                 rg[0:nt, k:k + W])
                            nc.vector.tensor_add(acc[0:nt], acc[0:nt], tmp[0:nt])
                        oh = fp.tile([128, W], f32, tag="oh")
                        nc.vector.tensor_mul(oh[0:nt], acc[0:nt], rcp[0:nt])
                        if ti == 0:
                            bc = fp.tile([16, W], f32, tag="bc")
                            for i in range(5):
                                nc.sync.dma_start(out=bc[i:i + 1, :],
                                                  in_=oh[5:6, :])
                            nc.vector.tensor_scalar_mul(
                                oh[0:5], oh[0:5], mk[0:5, 0:1])
                            nc.vector.tensor_scalar_mul(
                                bc[0:5], bc[0:5], mk[0:5, 1:2])
                            nc.vector.tensor_add(oh[0:5], oh[0:5], bc[0:5])
                        if ti == 2:
                            bc = fp.tile([16, W], f32, tag="bc")
                            nc.vector.memset(bc[0:5, :], 0.0)
                            for i in range(5):
                                nc.sync.dma_start(out=bc[5 + i:6 + i, :],
                                                  in_=oh[4:5, :])
                            nc.vector.tensor_scalar_mul(
                                oh[0:10], oh[0:10], mk[0:10, 2:3])
                            nc.vector.tensor_scalar_mul(
                                bc[0:10], bc[0:10], mk[0:10, 3:4])
                            nc.vector.tensor_add(oh[0:10], oh[0:10], bc[0:10])
                        nc.sync.dma_start(out=P[c, t0 - 6:t0 - 6 + nt, :],
                                          in_=oh[0:nt, :])

            # ---------------- vertical pass + blend ----------------
            with tc.tile_pool(name="vert", bufs=2) as vp:
                for r0 in (0, 128):
                    ev = vp.tile([128, K, W], f32, tag="ev")
                    for k in range(K):
                        nc.sync.dma_start(out=ev[:, k, :],
                                          in_=C[K + k, r0 + 12:r0 + 140, 1:1 + W])
                    sv = vp.tile([128, W], f32, tag="sv")
                    nc.vector.tensor_add(sv[:], ev[:, 0, :], ev[:, 1, :])
                    for k in range(2, K):
                        nc.vector.tensor_add(sv[:], sv[:], ev[:, k, :])
                    rcpv = vp.tile([128, W], f32, tag="rcpv")
                    nc.vector.reciprocal(rcpv[:], sv[:])
                    dep = vp.tile([128, W], f32, tag="dep")
                    nc.sync.dma_start(out=dep[:], in_=x[3, r0 + 12:r0 + 140, 1:1 + W])
                    msk = vp.tile([128, W], f32, tag="msk")
                    nc.scalar.activation(msk[:], dep[:], AF.Sigmoid,
                                         bias=cb[:], scale=15.0)
                    for c in range(3):
                        pt = vp.tile([128, K, W], f32, tag="pt")
                        for k in range(K):
                            nc.sync.dma_start(out=pt[:, k, :],
                                              in_=P[c, r0 + k:r0 + k + 128, :])
                        accv = vp.tile([128, W], f32, tag="accv")
                        tmpv = vp.tile([128, W], f32, tag="tmpv")
                        nc.vector.tensor_mul(accv[:], ev[:, 0, :], pt[:, 0, :])
                        for k in range(1, K):
                            nc.vector.tensor_mul(tmpv[:], ev[:, k, :], pt[:, k, :])
                            nc.vector.tensor_add(accv[:], accv[:], tmpv[:])
                        blur = vp.tile([128, W], f32, tag="blur")
                        nc.vector.tensor_mul(blur[:], accv[:], rcpv[:])
                        rgt = vp.tile([128, W], f32, tag="rgt")
                        nc.sync.dma_start(out=rgt[:],
                                          in_=x[c, r0 + 12:r0 + 140, 1:1 + W])
                        dt_ = vp.tile([128, W], f32, tag="dt")
                        nc.vector.tensor_sub(dt_[:], rgt[:], blur[:])
                        nc.vector.tensor_mul(dt_[:], dt_[:], msk[:])
                        ov = vp.tile([128, W], f32, tag="ov")
                        nc.vector.tensor_add(ov[:], blur[:], dt_[:])
                        nc.sync.dma_start(out=out[c, r0:r0 + 128, :], in_=ov[:])

    nc.compile()
    return nc


def _pack_weights(ins):
    f = np.float32
    w1, w2, w3, w4, w5 = ins["w1"], ins["w2"], ins["w3"], ins["w4"], ins["w5"]
    wcat = np.concatenate([ins["wh"], ins["wv"]], axis=0)  # [22,64,3,3]

    def packp3(w):  # [64,4,3,3] -> [12, 3*64]
        o = np.zeros((12, 3 * w.shape[0]), f)
        for dy in range(3):
            for dx in range(3):
                o[dy * 4:(dy + 1) * 4, dx * w.shape[0]:(dx + 1) * w.shape[0]] = \
                    w[:, :, dy, dx].T
        return o

    def packp2(w):  # [O,64,3,3] -> ([128,3O],[64,3O])
        O = w.shape[0]
        a = np.zeros((128, 3 * O), f)
        b = np.zeros((64, 3 * O), f)
        for dx in range(3):
            a[0:64, dx * O:(dx + 1) * O] = w[:, :, 0, dx].T
            a[64:128, dx * O:(dx + 1) * O] = w[:, :, 1, dx].T
            b[:, dx * O:(dx + 1) * O] = w[:, :, 2, dx].T
        return a, b

    def packf3(w):  # [O,128,3,3] -> [128, 9O]
        O = w.shape[0]
        a = np.zeros((128, 9 * O), f)
        for dy in range(3):
            for dx in range(3):
                a[:, (dy * 3 + dx) * O:(dy * 3 + dx + 1) * O] = w[:, :, dy, dx].T
        return a

    w2a, w2b = packp2(w2)
    w3a, w3b = packp2(w3)
    w6a, w6b = packp2(wcat)
    d = {"wl1": packp3(w1), "wl2a": w2a, "wl2b": w2b, "wl3a": w3a, "wl3b": w3b,
         "wl4": packf3(w4), "wl5": packf3(w5), "wl6a": w6a, "wl6b": w6b}
    for i, n in enumerate(["bl1", "bl2", "bl3", "bl4", "bl5"]):
        d[n] = np.ascontiguousarray(ins["b%d" % (i + 1)][:, None], f)
    d["bl6"] = np.ascontiguousarray(
        np.concatenate([ins["bh"], ins["bv"]])[:, None], f)
    return d


def _build_in_maps(inputs):
    rgb = np.asarray(inputs["rgb"], np.float32)
    depth = np.asarray(inputs["depth"], np.float32)
    wd = _pack_weights({k: np.asarray(v, np.float32) for k, v in inputs.items()
                        if k not in ("rgb", "depth")})

    x_full = np.concatenate([rgb, depth], axis=1)  # [B,4,H,W]
    rgb_pad = np.pad(rgb, ((0, 0), (0, 0), (11, 11), (5, 5)), mode="edge")

    in_maps = []
    for core in range(8):
        b, half = core // 2, core % 2
        s = half * RS
        xs = np.zeros((4, NBUF, WP), np.float32)
        lo, hi = s - 11, s + RS + 11
        clo, chi = max(lo, 0), min(hi, H)
        xs[:, 1 + (clo - lo):1 + (chi - lo), 1:1 + W] = x_full[b, :, clo:chi, :]
        rf = np.ascontiguousarray(rgb_pad[b, :, s:s + HS, :])
        keep_top = 0.0 if half == 0 else 1.0
        keep_bot = 0.0 if half == 1 else 1.0
        mrows = np.zeros((4, 5), np.float32)
        mrows[0, :] = keep_top
        mrows[1, :] = 1.0 - keep_top
        mrows[2, :] = keep_bot
        mrows[3, :] = 1.0 - keep_bot
        m = {"x": xs, "rgbf": rf, "masks": mrows}
        m.update(wd)
        in_maps.append(m)
    return in_maps


def _unshard(results):
    outp = np.zeros((B, 3, H, W), np.float32)
    for core in range(8):
        b, half = core // 2, core % 2
        s = half * RS
        outp[b, :, s:s + RS, :] = results[core]
    return outp


_exec_cache = None


def _get_exec():
    """Build the Bass program and a REUSABLE jitted dispatcher once.

    run_bass_via_pjrt builds a fresh jax.jit closure per call, so every
    warm call re-traces and re-lowers through XLA (~1.5s). Build the
    sharded executable a single time instead.
    """
    global _exec_cache
    if _exec_cache is not None:
        return _exec_cache

    import jax
    from jax.sharding import Mesh, PartitionSpec
    from concourse import mybir
    from concourse.bass2jax import (_bass_exec_p, partition_id_tensor,
                                    install_neuronx_cc_hook)

    nc = _build_program()
    install_neuronx_cc_hook()

    partition_name = (nc.partition_id_tensor.name
                      if nc.partition_id_tensor else None)
    in_names, out_names, out_avals, zero_shapes = [], [], [], []
    for alloc in nc.m.functions[0].allocations:
        if not isinstance(alloc, mybir.MemoryLocationSet):
            continue
        name = alloc.memorylocations[0].name
        if alloc.kind == "ExternalInput":
            if name != partition_name:
                in_names.append(name)
        elif alloc.kind == "ExternalOutput":
            shape = tuple(alloc.tensor_shape)
            dtype = mybir.dt.np(alloc.dtype)
            out_names.append(name)
            out_avals.append(jax.core.ShapedArray(shape, dtype))
            zero_shapes.append((shape, dtype))
    n_params = len(in_names)
    n_outs = len(out_avals)
    in_names_all = in_names + out_names
    if partition_name is not None:
        in_names_all.append(partition_name)
    donate = tuple(range(n_params, n_params + n_outs))

    def _body(*args):
        operands = list(args)
        if partition_name is not None:
            operands.append(partition_id_tensor())
        outs = _bass_exec_p.bind(
            *operands,
            out_avals=tuple(out_avals),
            in_names=tuple(in_names_all),
            out_names=tuple(out_names),
            lowering_input_output_aliases=(),
            sim_require_finite=True,
            sim_require_nnan=True,
            nc=nc,
        )
        return tuple(outs)

    devices = jax.devices()[:8]
    mesh = Mesh(np.asarray(devices), ("core",))
    in_specs = (PartitionSpec("core"),) * (n_params + n_outs)
    out_specs = (PartitionSpec("core"),) * len(out_names)
    try:
        from jax import shard_map
        smapped = shard_map(_body, mesh=mesh, in_specs=in_specs,
                            out_specs=out_specs, check_vma=False)
    except (ImportError, TypeError):
        from jax.experimental.shard_map import shard_map as shard_map_old
        smapped = shard_map_old(_body, mesh=mesh, in_specs=in_specs,
                                out_specs=out_specs, check_rep=False)
    sharded = jax.jit(smapped, donate_argnums=donate, keep_unused=True)
    _exec_cache = (nc, sharded, in_names, out_names, out_avals, zero_shapes)
    return _exec_cache


def kernel(**inputs):
    nc, sharded, in_names, out_names, out_avals, zero_shapes = _get_exec()
    in_maps = _build_in_maps(inputs)
    concat_in = [np.concatenate([np.asarray(m[name])[None] for m in in_maps],
                                axis=0).reshape(8 * in_maps[0][name].shape[0],
                                                *in_maps[0][name].shape[1:])
                 for name in in_names]
    concat_zeros = [np.zeros((8 * s[0], *s[1:]), dt) for s, dt in zero_shapes]
    out_arrs = sharded(*concat_in, *concat_zeros)
    oidx = out_names.index("out")
    res = np.asarray(out_arrs[oidx]).reshape(8, *out_avals[oidx].shape)
    return _unshard(res)


_last_exec_ns = None


def kernel_traced(**inputs):
    """Like kernel(), but captures an NTFF profile and sets _last_exec_ns to
    the per-core NEFF execution time (the true HW exec time)."""
    global _prog, _last_exec_ns
    import sys as _sys, types as _types
    if "antenv.axon_hooks" not in _sys.modules:
        hm = _types.ModuleType("antenv.axon_hooks")
        hm._hook = None
        hm.set_axon_ntff_profile_hook = lambda h: setattr(hm, "_hook", h)
        hm.get_axon_ntff_profile_hook = lambda: hm._hook
        _sys.modules["antenv.axon_hooks"] = hm
    from antenv.axon_hooks import set_axon_ntff_profile_hook
    from trn_agent_boot.trn_boot import _ntff_profile_via_ctypes
    set_axon_ntff_profile_hook(_ntff_profile_via_ctypes("/opt/axon/libaxon_pjrt.so"))
    import concourse.bass_utils as bu
    if not getattr(bu, "_upload_patched", False):
        bu.upload_artifacts = lambda tmpdir: "local://" + tmpdir
        bu._upload_patched = True
    if _prog is None:
        _prog = _build_program()
    in_maps = _build_in_maps(inputs)
    res = bu.run_bass_kernel_spmd(_prog, in_maps, core_ids=list(range(8)),
                                  trace=True, trace_cores=[0])
    if res.exec_time_ns is not None:
        _last_exec_ns = res.exec_time_ns
    trace_path = (res.instructions_and_trace[1]
                  if res.instructions_and_trace else None)
    return _unshard([res.results[c]["out"] for c in range(8)]), trace_path



# revision 7
# speedup vs baseline: 842.7132x; 1.5048x over previous
"""DFN Bokeh model on 8 TRN2 NeuronCores.

Sharding: 8 shards = (batch b, H-half) pairs; each core gets a 278-row slab
(256 out rows + 11-row conv halo each side, zero-padded at image edges by the
host). Conv chain runs as bf16 matmuls with channels on partitions and PSUM
row accumulation, ping-pong DRAM slabs in bf16. Layers with cout<=64 pack two
output rows into the PE array concurrently via column tiling (PSUM partition
halves); the two softmax-logit convs (cout=22) pack four rows via column
quads. The separable per-pixel filter runs with image rows on partitions; the
horizontal-pass output is bounced through a DRAM "plane" with replicate rows
so the vertical taps become plain row-offset DMA loads.
"""

import numpy as np
import sys

sys.path.insert(0, "/opt/trn_rl_repo")

B, H, W = 4, 512, 512
K = 11
RS = 256          # out rows per core
HS = RS + 22      # slab rows (conv halo 11 each side)
NBUF = HS + 2     # slab buffer rows incl. zero conv pad
WP = W + 2        # slab cols incl. zero conv pad
RB = 8            # conv rows per block (PSUM banks)
# all blocks are a full RB rows; the last block overlaps the previous one
# (recomputes 2 rows) so pair/quad packing never sees a ragged block
BLOCKS = list(range(0, HS - RB, RB)) + [HS - RB]

_prog = None


def _build_program():
    import concourse.bacc as bacc
    import concourse.mybir as mybir
    from concourse import tile

    f32 = mybir.dt.float32
    bf16 = mybir.dt.bfloat16
    AF = mybir.ActivationFunctionType

    nc = bacc.Bacc("TRN2", target_bir_lowering=False, debug=False, num_devices=8)

    x = nc.dram_tensor("x", [4, NBUF, WP], f32, kind="ExternalInput").ap()
    xdy = nc.dram_tensor("xdy", [12, NBUF, WP], bf16, kind="ExternalInput").ap()
    rgbf = nc.dram_tensor("rgbf", [3, HS, W + 10], f32, kind="ExternalInput").ap()
    masks = nc.dram_tensor("masks", [4, 5], f32, kind="ExternalInput").ap()
    wnames = {
        "wl1": (12, 192), "wl2a": (128, 192), "wl2b": (128, 192),
        "wl3a": (128, 384), "wl3b": (64, 384), "wl4": (128, 1152),
        "wl5": (128, 576), "wl6a": (128, 66), "wl6b": (128, 66),
    }
    wd = {n: nc.dram_tensor(n, list(s), bf16, kind="ExternalInput").ap()
          for n, s in wnames.items()}
    bnames = ["bl1", "bl2", "bl3", "bl4", "bl5", "bl6"]
    bd = {n: nc.dram_tensor(n, [128, 1], f32, kind="ExternalInput").ap()
          for n in bnames}
    out = nc.dram_tensor("out", [3, RS, W], f32, kind="ExternalOutput").ap()

    with tile.TileContext(nc) as tc:
        with (
            tc.tile_pool(name="dram", bufs=1, space="DRAM") as dpool,
            tc.tile_pool(name="wts", bufs=1) as wpool,
        ):
            A = dpool.tile([128, NBUF, WP], bf16, tag="A")
            Bd = dpool.tile([128, NBUF, WP], bf16, tag="B")
            C = dpool.tile([22, NBUF, WP], f32, tag="C")
            P = dpool.tile([3, HS - 12 + 10, W], f32, tag="P")  # 266 rows

            wt = {}
            for n, s in wnames.items():
                t = wpool.tile(list(s), bf16, tag=n)
                nc.sync.dma_start(out=t[:], in_=wd[n][:])
                wt[n] = t
            bt = {}
            for n in bnames:
                t = wpool.tile([128, 1], f32, tag=n)
                nc.sync.dma_start(out=t[:], in_=bd[n][:])
                bt[n] = t
            # mk cols: 0=keep_top 1=rep_top (partitions 0-4),
            #          2=keep_bot 3=rep_bot (partitions 5-9)
            mk = wpool.tile([16, 4], f32, tag="mk")
            nc.vector.memset(mk[0:5, 2:3], 1.0)
            nc.vector.memset(mk[0:5, 3:4], 0.0)
            nc.sync.dma_start(out=mk[0:5, 0:1], in_=masks[0:1, :])
            nc.sync.dma_start(out=mk[0:5, 1:2], in_=masks[1:2, :])
            nc.sync.dma_start(out=mk[5:10, 2:3], in_=masks[2:3, :])
            nc.sync.dma_start(out=mk[5:10, 3:4], in_=masks[3:4, :])

            cb = wpool.tile([128, 1], f32, tag="cb")
            nc.vector.memset(cb[:], -3.0)

            # zero the pad rows of A and B
            with tc.tile_pool(name="zz", bufs=1) as zpool:
                z = zpool.tile([128, WP], bf16, tag="z")
                nc.vector.memset(z[:], 0.0)
                for dst in (A, Bd):
                    nc.sync.dma_start(out=dst[:, 0, :], in_=z[:])
                    nc.sync.dma_start(out=dst[:, NBUF - 1, :], in_=z[:])

            # ---------------- conv chain ----------------
            def mm(pp, lhsT, rhs, start, stop, tp=None):
                nc.tensor.matmul(pp, lhsT=lhsT, rhs=rhs, start=start, stop=stop,
                                 tile_position=tp)

            def store_pairs(dst, r0, ot, cout=64):
                nc.sync.dma_start(out=dst[0:cout, r0 + 1:r0 + 1 + RB:2, :],
                                  in_=ot[0:cout, :, :])
                nc.sync.dma_start(out=dst[0:cout, r0 + 2:r0 + 2 + RB:2, :],
                                  in_=ot[64:64 + cout, :, :])

            def conv_l1(src, dst, cpool, opool, ppool):
                # k=12 (ch,dy packed by host), cout=64, col-paired rows
                for r0 in BLOCKS:
                    it = cpool.tile([16, RB, WP], bf16, tag="cin1")
                    nc.sync.dma_start(out=it[0:12, 0:RB, :],
                                      in_=src[0:12, r0 + 1:r0 + 1 + RB, :])
                    ot = opool.tile([128, RB // 2, WP], bf16, tag="cout1")
                    nc.vector.memset(ot[:, :, 0:1], 0.0)
                    nc.vector.memset(ot[:, :, WP - 1:WP], 0.0)
                    for q in range(RB // 2):
                        rr = 2 * q
                        pp = ppool.tile([128, W], f32, tag="pp", name="p1%d" % q)
                        for dx in range(3):
                            lt = wt["wl1"][0:12, dx * 64:(dx + 1) * 64]
                            mm(pp[0:64], lt, it[0:12, rr, dx:dx + W],
                               dx == 0, dx == 2)
                            mm(pp[64:128], lt, it[0:12, rr + 1, dx:dx + W],
                               dx == 0, dx == 2)
                        nc.scalar.activation(ot[:, q, 1:1 + W], pp[:], AF.Relu,
                                             bias=bt["bl1"][:])
                    store_pairs(dst, r0, ot)

            def conv_p2c(src, dst, ka, kb, bkey, cpool, opool, ppool):
                # cin=64 (dy01 packed + dy2), cout=64, col-paired rows
                for r0 in BLOCKS:
                    it = cpool.tile([128, RB + 2, WP], bf16, tag="cin")
                    nc.sync.dma_start(out=it[0:64, 0:RB + 2, :],
                                      in_=src[0:64, r0:r0 + RB + 2, :])
                    nc.sync.dma_start(out=it[64:128, 0:RB + 1, :],
                                      in_=src[0:64, r0 + 1:r0 + RB + 2, :])
                    ot = opool.tile([128, RB // 2, WP], bf16, tag="cout")
                    nc.vector.memset(ot[:, :, 0:1], 0.0)
                    nc.vector.memset(ot[:, :, WP - 1:WP], 0.0)
                    for q in range(RB // 2):
                        rr = 2 * q
                        pp = ppool.tile([128, W], f32, tag="pp", name="p2%d" % q)
                        for dx in range(3):
                            lt = wt[ka][0:128, dx * 64:(dx + 1) * 64]
                            mm(pp[0:64], lt, it[0:128, rr, dx:dx + W],
                               dx == 0, False)
                            mm(pp[64:128], lt, it[0:128, rr + 1, dx:dx + W],
                               dx == 0, False)
                        for dx in range(3):
                            mm(pp[0:64], wt[kb][0:64, dx * 64:(dx + 1) * 64],
                               it[0:64, rr + 2, dx:dx + W], False, dx == 2)
                            mm(pp[64:128], wt[kb][64:128, dx * 64:(dx + 1) * 64],
                               it[64:128, rr + 2, dx:dx + W], False, dx == 2)
                        nc.scalar.activation(ot[:, q, 1:1 + W], pp[:], AF.Relu,
                                             bias=bt[bkey][:])
                    store_pairs(dst, r0, ot)

            def conv_p2f(src, dst, ka, kb, bkey, cpool, opool, ppool):
                # cin=64 (dy01 packed + dy2), cout=128
                for r0 in BLOCKS:
                    it = cpool.tile([128, RB + 2, WP], bf16, tag="cin")
                    nc.sync.dma_start(out=it[0:64, 0:RB + 2, :],
                                      in_=src[0:64, r0:r0 + RB + 2, :])
                    nc.sync.dma_start(out=it[64:128, 0:RB, :],
                                      in_=src[0:64, r0 + 1:r0 + RB + 1, :])
                    ot = opool.tile([128, RB, WP], bf16, tag="coutf")
                    nc.vector.memset(ot[:, :, 0:1], 0.0)
                    nc.vector.memset(ot[:, :, WP - 1:WP], 0.0)
                    for rr in range(RB):
                        pp = ppool.tile([128, W], f32, tag="pp", name="p3%d" % rr)
                        for dx in range(3):
                            mm(pp[:], wt[ka][0:128, dx * 128:(dx + 1) * 128],
                               it[0:128, rr, dx:dx + W], dx == 0, False)
                        for dx in range(3):
                            mm(pp[:], wt[kb][0:64, dx * 128:(dx + 1) * 128],
                               it[0:64, rr + 2, dx:dx + W], False, dx == 2)
                        nc.scalar.activation(ot[:, rr, 1:1 + W], pp[:], AF.Relu,
                                             bias=bt[bkey][:])
                    nc.sync.dma_start(out=dst[0:128, r0 + 1:r0 + 1 + RB, :],
                                      in_=ot[:, :, :])

            def conv_f3(src, dst, wkey, bkey, cpool, opool, ppool):
                # cin=128, cout=128
                for r0 in BLOCKS:
                    it = cpool.tile([128, RB + 2, WP], bf16, tag="cin")
                    nc.sync.dma_start(out=it[0:128, 0:RB + 2, :],
                                      in_=src[0:128, r0:r0 + RB + 2, :])
                    ot = opool.tile([128, RB, WP], bf16, tag="coutf")
                    nc.vector.memset(ot[:, :, 0:1], 0.0)
                    nc.vector.memset(ot[:, :, WP - 1:WP], 0.0)
                    for rr in range(RB):
                        pp = ppool.tile([128, W], f32, tag="pp", name="p4%d" % rr)
                        for dy in range(3):
                            for dx in range(3):
                                g = dy * 3 + dx
                                mm(pp[:], wt[wkey][:, g * 128:(g + 1) * 128],
                                   it[0:128, rr + dy, dx:dx + W],
                                   g == 0, g == 8)
                        nc.scalar.activation(ot[:, rr, 1:1 + W], pp[:], AF.Relu,
                                             bias=bt[bkey][:])
                    nc.sync.dma_start(out=dst[0:128, r0 + 1:r0 + 1 + RB, :],
                                      in_=ot[:, :, :])

            def conv_f3c(src, dst, wkey, bkey, cpool, opool, ppool):
                # cin=128, cout=64, col-paired rows
                for r0 in BLOCKS:
                    it = cpool.tile([128, RB + 2, WP], bf16, tag="cin")
                    nc.sync.dma_start(out=it[0:128, 0:RB + 2, :],
                                      in_=src[0:128, r0:r0 + RB + 2, :])
                    ot = opool.tile([128, RB // 2, WP], bf16, tag="cout")
                    nc.vector.memset(ot[:, :, 0:1], 0.0)
                    nc.vector.memset(ot[:, :, WP - 1:WP], 0.0)
                    for q in range(RB // 2):
                        rr = 2 * q
                        pp = ppool.tile([128, W], f32, tag="pp", name="p5%d" % q)
                        for dy in range(3):
                            for dx in range(3):
                                g = dy * 3 + dx
                                lt = wt[wkey][:, g * 64:(g + 1) * 64]
                                mm(pp[0:64], lt, it[0:128, rr + dy, dx:dx + W],
                                   g == 0, g == 8)
                                mm(pp[64:128], lt,
                                   it[0:128, rr + 1 + dy, dx:dx + W],
                                   g == 0, g == 8)
                        nc.scalar.activation(ot[:, q, 1:1 + W], pp[:], AF.Relu,
                                             bias=bt[bkey][:])
                    store_pairs(dst, r0, ot)

            def conv_p2q(src, dst, ka, kb, bkey, cpool, opool, ppool):
                # cin=64, cout=22, col-quad rows, Exp activation
                for r0 in BLOCKS:
                    it = cpool.tile([128, RB + 2, WP], bf16, tag="cin")
                    nc.sync.dma_start(out=it[0:64, 0:RB + 2, :],
                                      in_=src[0:64, r0:r0 + RB + 2, :])
                    nc.sync.dma_start(out=it[64:128, 0:RB + 1, :],
                                      in_=src[0:64, r0 + 1:r0 + RB + 2, :])
                    ot = opool.tile([128, RB // 4, WP], f32, tag="cout6")
                    for q in range(RB // 4):
                        rr = 4 * q
                        pp = ppool.tile([128, W], f32, tag="pp", name="p6%d" % q)
                        for dx in range(3):
                            lt = wt[ka][0:128, dx * 22:(dx + 1) * 22]
                            mm(pp[0:22], lt, it[0:128, rr, dx:dx + W],
                               dx == 0, False, (0, 0))
                            mm(pp[32:54], lt, it[0:128, rr + 1, dx:dx + W],
                               dx == 0, False, (0, 32))
                            mm(pp[64:86], lt, it[0:128, rr + 2, dx:dx + W],
                               dx == 0, False, (0, 64))
                            mm(pp[96:118], lt, it[0:128, rr + 3, dx:dx + W],
                               dx == 0, False, (0, 96))
                        for dx in range(3):
                            la = wt[kb][0:64, dx * 22:(dx + 1) * 22]
                            lb = wt[kb][64:128, dx * 22:(dx + 1) * 22]
                            mm(pp[0:22], la, it[0:64, rr + 2, dx:dx + W],
                               False, dx == 2, (0, 0))
                            mm(pp[32:54], la, it[0:64, rr + 3, dx:dx + W],
                               False, dx == 2, (0, 32))
                            mm(pp[64:86], lb, it[64:128, rr + 3, dx:dx + W],
                               False, dx == 2, (64, 64))
                            mm(pp[96:118], lb, it[64:128, rr + 4, dx:dx + W],
                               False, dx == 2, (64, 96))
                        nc.scalar.activation(ot[0:118, q, 1:1 + W], pp[0:118],
                                             AF.Exp, bias=bt[bkey][0:118])
                    for g in range(4):
                        nc.sync.dma_start(
                            out=dst[0:22, r0 + 1 + g:r0 + 1 + RB:4, 1:1 + W],
                            in_=ot[32 * g:32 * g + 22, 0:RB // 4, 1:1 + W])

            with (
                tc.tile_pool(name="cin", bufs=3) as cpool,
                tc.tile_pool(name="cout", bufs=3) as opool,
                tc.tile_pool(name="psum", bufs=8, space="PSUM") as ppool,
            ):
                conv_l1(xdy, A, cpool, opool, ppool)
                conv_p2c(A, Bd, "wl2a", "wl2b", "bl2", cpool, opool, ppool)
                conv_p2f(Bd, A, "wl3a", "wl3b", "bl3", cpool, opool, ppool)
                conv_f3(A, Bd, "wl4", "bl4", cpool, opool, ppool)
                conv_f3c(Bd, A, "wl5", "bl5", cpool, opool, ppool)
                conv_p2q(A, C, "wl6a", "wl6b", "bl6", cpool, opool, ppool)

            # ---------------- horizontal pass ----------------
            with tc.tile_pool(name="filt", bufs=2) as fp:
                tts = [(6, 128, 0), (134, 128, 1), (262, 10, 2)]
                for t0, nt, ti in tts:
                    ex = fp.tile([128, K, W], f32, tag="exh")
                    for k in range(K):
                        nc.sync.dma_start(out=ex[0:nt, k, :],
                                          in_=C[k, t0 + 1:t0 + 1 + nt, 1:1 + W])
                    sh = fp.tile([128, W], f32, tag="sh")
                    nc.vector.tensor_add(sh[0:nt], ex[0:nt, 0, :], ex[0:nt, 1, :])
                    for k in range(2, K):
                        nc.vector.tensor_add(sh[0:nt], sh[0:nt], ex[0:nt, k, :])
                    rcp = fp.tile([128, W], f32, tag="rcp")
                    nc.vector.reciprocal(rcp[0:nt], sh[0:nt])
                    for c in range(3):
                        rg = fp.tile([128, W + 10], f32, tag="rg")
                        nc.sync.dma_start(out=rg[0:nt, :],
                                          in_=rgbf[c, t0:t0 + nt, :])
                        acc = fp.tile([128, W], f32, tag="acc")
                        tmp = fp.tile([128, W], f32, tag="tmp")
                        nc.vector.tensor_mul(acc[0:nt], ex[0:nt, 0, :],
                                             rg[0:nt, 0:W])
                        for k in range(1, K):
                            nc.vector.tensor_mul(tmp[0:nt], ex[0:nt, k, :],
                                                 rg[0:nt, k:k + W])
                            nc.vector.tensor_add(acc[0:nt], acc[0:nt], tmp[0:nt])
                        oh = fp.tile([128, W], f32, tag="oh")
                        nc.vector.tensor_mul(oh[0:nt], acc[0:nt], rcp[0:nt])
                        if ti == 0:
                            bc = fp.tile([16, W], f32, tag="bc")
                            for i in range(5):
                                nc.sync.dma_start(out=bc[i:i + 1, :],
                                                  in_=oh[5:6, :])
                            nc.vector.tensor_scalar_mul(
                                oh[0:5], oh[0:5], mk[0:5, 0:1])
                            nc.vector.tensor_scalar_mul(
                                bc[0:5], bc[0:5], mk[0:5, 1:2])
                            nc.vector.tensor_add(oh[0:5], oh[0:5], bc[0:5])
                        if ti == 2:
                            bc = fp.tile([16, W], f32, tag="bc")
                            nc.vector.memset(bc[0:5, :], 0.0)
                            for i in range(5):
                                nc.sync.dma_start(out=bc[5 + i:6 + i, :],
                                                  in_=oh[4:5, :])
                            nc.vector.tensor_scalar_mul(
                                oh[0:10], oh[0:10], mk[0:10, 2:3])
                            nc.vector.tensor_scalar_mul(
                                bc[0:10], bc[0:10], mk[0:10, 3:4])
                            nc.vector.tensor_add(oh[0:10], oh[0:10], bc[0:10])
                        nc.sync.dma_start(out=P[c, t0 - 6:t0 - 6 + nt, :],
                                          in_=oh[0:nt, :])

            # ---------------- vertical pass + blend ----------------
            with tc.tile_pool(name="vert", bufs=2) as vp:
                for r0 in (0, 128):
                    ev = vp.tile([128, K, W], f32, tag="ev")
                    for k in range(K):
                        nc.sync.dma_start(out=ev[:, k, :],
                                          in_=C[K + k, r0 + 12:r0 + 140, 1:1 + W])
                    sv = vp.tile([128, W], f32, tag="sv")
                    nc.vector.tensor_add(sv[:], ev[:, 0, :], ev[:, 1, :])
                    for k in range(2, K):
                        nc.vector.tensor_add(sv[:], sv[:], ev[:, k, :])
                    rcpv = vp.tile([128, W], f32, tag="rcpv")
                    nc.vector.reciprocal(rcpv[:], sv[:])
                    dep = vp.tile([128, W], f32, tag="dep")
                    nc.sync.dma_start(out=dep[:], in_=x[3, r0 + 12:r0 + 140, 1:1 + W])
                    msk = vp.tile([128, W], f32, tag="msk")
                    nc.scalar.activation(msk[:], dep[:], AF.Sigmoid,
                                         bias=cb[:], scale=15.0)
                    for c in range(3):
                        pt = vp.tile([128, K, W], f32, tag="pt")
                        for k in range(K):
                            nc.sync.dma_start(out=pt[:, k, :],
                                              in_=P[c, r0 + k:r0 + k + 128, :])
                        accv = vp.tile([128, W], f32, tag="accv")
                        tmpv = vp.tile([128, W], f32, tag="tmpv")
                        nc.vector.tensor_mul(accv[:], ev[:, 0, :], pt[:, 0, :])
                        for k in range(1, K):
                            nc.vector.tensor_mul(tmpv[:], ev[:, k, :], pt[:, k, :])
                            nc.vector.tensor_add(accv[:], accv[:], tmpv[:])
                        blur = vp.tile([128, W], f32, tag="blur")
                        nc.vector.tensor_mul(blur[:], accv[:], rcpv[:])
                        rgt = vp.tile([128, W], f32, tag="rgt")
                        nc.sync.dma_start(out=rgt[:],
                                          in_=x[c, r0 + 12:r0 + 140, 1:1 + W])
                        dt_ = vp.tile([128, W], f32, tag="dt")
                        nc.vector.tensor_sub(dt_[:], rgt[:], blur[:])
                        nc.vector.tensor_mul(dt_[:], dt_[:], msk[:])
                        ov = vp.tile([128, W], f32, tag="ov")
                        nc.vector.tensor_add(ov[:], blur[:], dt_[:])
                        nc.sync.dma_start(out=out[c, r0:r0 + 128, :], in_=ov[:])

    nc.compile()
    return nc


def _pack_weights(ins):
    import ml_dtypes
    bf = ml_dtypes.bfloat16
    f = np.float32
    w1, w2, w3, w4, w5 = ins["w1"], ins["w2"], ins["w3"], ins["w4"], ins["w5"]
    wcat = np.concatenate([ins["wh"], ins["wv"]], axis=0)  # [22,64,3,3]

    def packl1(w):  # [64,4,3,3] -> [12, 3*64]: [dy*4+ch, dx*64+o]
        o = np.zeros((12, 192), f)
        for dy in range(3):
            for dx in range(3):
                o[dy * 4:(dy + 1) * 4, dx * 64:(dx + 1) * 64] = w[:, :, dy, dx].T
        return o

    def packp2(w):  # [O,64,3,3] -> ([128,3O] dy01, [64,3O] dy2)
        O = w.shape[0]
        a = np.zeros((128, 3 * O), f)
        b = np.zeros((64, 3 * O), f)
        for dx in range(3):
            a[0:64, dx * O:(dx + 1) * O] = w[:, :, 0, dx].T
            a[64:128, dx * O:(dx + 1) * O] = w[:, :, 1, dx].T
            b[:, dx * O:(dx + 1) * O] = w[:, :, 2, dx].T
        return a, b

    def packf3(w):  # [O,128,3,3] -> [128, 9O]
        O = w.shape[0]
        a = np.zeros((128, 9 * O), f)
        for dy in range(3):
            for dx in range(3):
                a[:, (dy * 3 + dx) * O:(dy * 3 + dx + 1) * O] = w[:, :, dy, dx].T
        return a

    def dup2(b):  # [64,3O] -> [128,3O] (both halves)
        return np.concatenate([b, b], axis=0)

    w2a, w2b = packp2(w2)
    w3a, w3b = packp2(w3)
    w6a, w6b = packp2(wcat)
    d = {"wl1": packl1(w1), "wl2a": w2a, "wl2b": dup2(w2b),
         "wl3a": w3a, "wl3b": w3b,
         "wl4": packf3(w4), "wl5": packf3(w5),
         "wl6a": w6a, "wl6b": dup2(w6b)}
    d = {k: np.ascontiguousarray(v.astype(bf)) for k, v in d.items()}

    def bias128(b, dup):
        o = np.zeros((128, 1), f)
        o[0:len(b), 0] = b
        if dup:
            o[64:64 + len(b), 0] = b
        return o

    d["bl1"] = bias128(ins["b1"], True)
    d["bl2"] = bias128(ins["b2"], True)
    d["bl3"] = bias128(ins["b3"], False)
    d["bl4"] = bias128(ins["b4"], False)
    d["bl5"] = bias128(ins["b5"], True)
    b6 = np.concatenate([ins["bh"], ins["bv"]])
    o6 = np.zeros((128, 1), f)
    for g in range(4):
        o6[32 * g:32 * g + 22, 0] = b6
    d["bl6"] = o6
    return d


def _build_in_maps(inputs):
    import ml_dtypes
    bf = ml_dtypes.bfloat16
    rgb = np.asarray(inputs["rgb"], np.float32)
    depth = np.asarray(inputs["depth"], np.float32)
    wd = _pack_weights({k: np.asarray(v, np.float32) for k, v in inputs.items()
                        if k not in ("rgb", "depth")})

    x_full = np.concatenate([rgb, depth], axis=1)  # [B,4,H,W]
    rgb_pad = np.pad(rgb, ((0, 0), (0, 0), (11, 11), (5, 5)), mode="edge")

    in_maps = []
    for core in range(8):
        b, half = core // 2, core % 2
        s = half * RS
        xs = np.zeros((4, NBUF, WP), np.float32)
        lo, hi = s - 11, s + RS + 11
        clo, chi = max(lo, 0), min(hi, H)
        xs[:, 1 + (clo - lo):1 + (chi - lo), 1:1 + W] = x_full[b, :, clo:chi, :]
        # dy-packed bf16 conv input: xdy[dy*4+ch, j, c] = xs[ch, j+dy-1, c]
        xsb = xs.astype(bf)
        xdy = np.zeros((12, NBUF, WP), bf)
        xdy[0:4, 1:NBUF, :] = xsb[:, 0:NBUF - 1, :]
        xdy[4:8, :, :] = xsb
        xdy[8:12, 0:NBUF - 1, :] = xsb[:, 1:NBUF, :]
        rf = np.ascontiguousarray(rgb_pad[b, :, s:s + HS, :])
        keep_top = 0.0 if half == 0 else 1.0
        keep_bot = 0.0 if half == 1 else 1.0
        mrows = np.zeros((4, 5), np.float32)
        mrows[0, :] = keep_top
        mrows[1, :] = 1.0 - keep_top
        mrows[2, :] = keep_bot
        mrows[3, :] = 1.0 - keep_bot
        m = {"x": xs, "xdy": xdy, "rgbf": rf, "masks": mrows}
        m.update(wd)
        in_maps.append(m)
    return in_maps


def _unshard(results):
    outp = np.zeros((B, 3, H, W), np.float32)
    for core in range(8):
        b, half = core // 2, core % 2
        s = half * RS
        outp[b, :, s:s + RS, :] = results[core]
    return outp


_exec_cache = None


def _get_exec():
    """Build the Bass program and a REUSABLE jitted dispatcher once.

    run_bass_via_pjrt builds a fresh jax.jit closure per call, so every
    warm call re-traces and re-lowers through XLA (~1.5s). Build the
    sharded executable a single time instead.
    """
    global _exec_cache
    if _exec_cache is not None:
        return _exec_cache

    import jax
    from jax.sharding import Mesh, PartitionSpec
    from concourse import mybir
    from concourse.bass2jax import (_bass_exec_p, partition_id_tensor,
                                    install_neuronx_cc_hook)

    nc = _build_program()
    install_neuronx_cc_hook()

    partition_name = (nc.partition_id_tensor.name
                      if nc.partition_id_tensor else None)
    in_names, out_names, out_avals, zero_shapes = [], [], [], []
    for alloc in nc.m.functions[0].allocations:
        if not isinstance(alloc, mybir.MemoryLocationSet):
            continue
        name = alloc.memorylocations[0].name
        if alloc.kind == "ExternalInput":
            if name != partition_name:
                in_names.append(name)
        elif alloc.kind == "ExternalOutput":
            shape = tuple(alloc.tensor_shape)
            dtype = mybir.dt.np(alloc.dtype)
            out_names.append(name)
            out_avals.append(jax.core.ShapedArray(shape, dtype))
            zero_shapes.append((shape, dtype))
    n_params = len(in_names)
    n_outs = len(out_avals)
    in_names_all = in_names + out_names
    if partition_name is not None:
        in_names_all.append(partition_name)
    donate = tuple(range(n_params, n_params + n_outs))

    def _body(*args):
        operands = list(args)
        if partition_name is not None:
            operands.append(partition_id_tensor())
        outs = _bass_exec_p.bind(
            *operands,
            out_avals=tuple(out_avals),
            in_names=tuple(in_names_all),
            out_names=tuple(out_names),
            lowering_input_output_aliases=(),
            sim_require_finite=True,
            sim_require_nnan=True,
            nc=nc,
        )
        return tuple(outs)

    devices = jax.devices()[:8]
    mesh = Mesh(np.asarray(devices), ("core",))
    in_specs = (PartitionSpec("core"),) * (n_params + n_outs)
    out_specs = (PartitionSpec("core"),) * len(out_names)
    try:
        from jax import shard_map
        smapped = shard_map(_body, mesh=mesh, in_specs=in_specs,
                            out_specs=out_specs, check_vma=False)
    except (ImportError, TypeError):
        from jax.experimental.shard_map import shard_map as shard_map_old
        smapped = shard_map_old(_body, mesh=mesh, in_specs=in_specs,
                                out_specs=out_specs, check_rep=False)
    sharded = jax.jit(smapped, donate_argnums=donate, keep_unused=True)
    _exec_cache = (nc, sharded, in_names, out_names, out_avals, zero_shapes)
    return _exec_cache


def kernel(**inputs):
    nc, sharded, in_names, out_names, out_avals, zero_shapes = _get_exec()
    in_maps = _build_in_maps(inputs)
    concat_in = [np.concatenate([np.asarray(m[name])[None] for m in in_maps],
                                axis=0).reshape(8 * in_maps[0][name].shape[0],
                                                *in_maps[0][name].shape[1:])
                 for name in in_names]
    concat_zeros = [np.zeros((8 * s[0], *s[1:]), dt) for s, dt in zero_shapes]
    out_arrs = sharded(*concat_in, *concat_zeros)
    oidx = out_names.index("out")
    res = np.asarray(out_arrs[oidx]).reshape(8, *out_avals[oidx].shape)
    return _unshard(res)


_last_exec_ns = None


def kernel_traced(**inputs):
    """Like kernel(), but captures an NTFF profile and sets _last_exec_ns to
    the per-core NEFF execution time (the true HW exec time)."""
    global _prog, _last_exec_ns
    import sys as _sys, types as _types
    if "antenv.axon_hooks" not in _sys.modules:
        hm = _types.ModuleType("antenv.axon_hooks")
        hm._hook = None
        hm.set_axon_ntff_profile_hook = lambda h: setattr(hm, "_hook", h)
        hm.get_axon_ntff_profile_hook = lambda: hm._hook
        _sys.modules["antenv.axon_hooks"] = hm
    from antenv.axon_hooks import set_axon_ntff_profile_hook
    from trn_agent_boot.trn_boot import _ntff_profile_via_ctypes
    set_axon_ntff_profile_hook(_ntff_profile_via_ctypes("/opt/axon/libaxon_pjrt.so"))
    import concourse.bass_utils as bu
    if not getattr(bu, "_upload_patched", False):
        bu.upload_artifacts = lambda tmpdir: "local://" + tmpdir
        bu._upload_patched = True
    if _prog is None:
        _prog = _build_program()
    in_maps = _build_in_maps(inputs)
    res = bu.run_bass_kernel_spmd(_prog, in_maps, core_ids=list(range(8)),
                                  trace=True, trace_cores=[0])
    if res.exec_time_ns is not None:
        _last_exec_ns = res.exec_time_ns
    trace_path = (res.instructions_and_trace[1]
                  if res.instructions_and_trace else None)
    return _unshard([res.results[c]["out"] for c in range(8)]), trace_path


# revision 8
# speedup vs baseline: 997.0201x; 1.1831x over previous
"""DFN Bokeh model on 8 TRN2 NeuronCores.

Sharding: 8 shards = (batch b, H-half) pairs; each core gets a 278-row slab
(256 out rows + 11-row conv halo each side, zero-padded at image edges by the
host). Conv chain runs as bf16 matmuls with channels on partitions and PSUM
row accumulation, ping-pong DRAM slabs in bf16. Layers with cout<=64 pack two
output rows into the PE array concurrently via column tiling (PSUM partition
halves); the two softmax-logit convs (cout=22) pack four rows via column
quads. The separable per-pixel filter runs with image rows on partitions; the
horizontal-pass output is bounced through a DRAM "plane" with replicate rows
so the vertical taps become plain row-offset DMA loads.
"""

import numpy as np
import sys

sys.path.insert(0, "/opt/trn_rl_repo")

B, H, W = 4, 512, 512
K = 11
RS = 256          # out rows per core
HS = RS + 22      # slab rows (conv halo 11 each side)
NBUF = HS + 3     # slab rows incl. zero conv pad (+1 for dual-half reads)
WP = W + 2        # slab cols incl. zero conv pad
RB = 8            # conv rows per block (PSUM banks)
# all blocks are a full RB rows; the last block overlaps the previous one
# (recomputes 2 rows) so pair/quad packing never sees a ragged block
BLOCKS = list(range(0, HS - RB, RB)) + [HS - RB]

_prog = None


def _build_program():
    import concourse.bacc as bacc
    import concourse.mybir as mybir
    from concourse import tile

    f32 = mybir.dt.float32
    bf16 = mybir.dt.bfloat16
    AF = mybir.ActivationFunctionType

    nc = bacc.Bacc("TRN2", target_bir_lowering=False, debug=False, num_devices=8)

    x = nc.dram_tensor("x", [4, NBUF, WP], f32, kind="ExternalInput").ap()
    xdy = nc.dram_tensor("xdy", [12, NBUF, WP], bf16, kind="ExternalInput").ap()
    rgbf = nc.dram_tensor("rgbf", [3, HS, W + 10], f32, kind="ExternalInput").ap()
    masks = nc.dram_tensor("masks", [4, 5], f32, kind="ExternalInput").ap()
    wnames = {
        "wl1": (12, 192), "wl2a": (128, 192), "wl2b": (128, 192),
        "wl3a": (128, 384), "wl3b": (128, 384), "wl4": (128, 1152),
        "wl5": (128, 576), "wl6a": (128, 66), "wl6b": (128, 66),
    }
    wd = {n: nc.dram_tensor(n, list(s), bf16, kind="ExternalInput").ap()
          for n, s in wnames.items()}
    bnames = ["bl1", "bl2", "bl3", "bl4", "bl5", "bl6"]
    bd = {n: nc.dram_tensor(n, [128, 1], f32, kind="ExternalInput").ap()
          for n in bnames}
    out = nc.dram_tensor("out", [3, RS, W], f32, kind="ExternalOutput").ap()

    with tile.TileContext(nc) as tc:
        with (
            tc.tile_pool(name="dram", bufs=1, space="DRAM") as dpool,
            tc.tile_pool(name="wts", bufs=1) as wpool,
        ):
            A = dpool.tile([128, NBUF, WP], bf16, tag="A")
            Bd = dpool.tile([128, NBUF, WP], bf16, tag="B")
            C = dpool.tile([22, NBUF, WP], f32, tag="C")
            P = dpool.tile([3, HS - 12 + 10, W], f32, tag="P")  # 266 rows

            wt = {}
            for n, s in wnames.items():
                t = wpool.tile(list(s), bf16, tag=n)
                nc.sync.dma_start(out=t[:], in_=wd[n][:])
                wt[n] = t
            bt = {}
            for n in bnames:
                t = wpool.tile([128, 1], f32, tag=n)
                nc.sync.dma_start(out=t[:], in_=bd[n][:])
                bt[n] = t
            # mk cols: 0=keep_top 1=rep_top (partitions 0-4),
            #          2=keep_bot 3=rep_bot (partitions 5-9)
            mk = wpool.tile([16, 4], f32, tag="mk")
            nc.vector.memset(mk[0:5, 2:3], 1.0)
            nc.vector.memset(mk[0:5, 3:4], 0.0)
            nc.sync.dma_start(out=mk[0:5, 0:1], in_=masks[0:1, :])
            nc.sync.dma_start(out=mk[0:5, 1:2], in_=masks[1:2, :])
            nc.sync.dma_start(out=mk[5:10, 2:3], in_=masks[2:3, :])
            nc.sync.dma_start(out=mk[5:10, 3:4], in_=masks[3:4, :])

            cb = wpool.tile([128, 1], f32, tag="cb")
            nc.vector.memset(cb[:], -3.0)

            # zero the pad rows of A and B
            with tc.tile_pool(name="zz", bufs=1) as zpool:
                z = zpool.tile([128, WP], bf16, tag="z")
                nc.vector.memset(z[:], 0.0)
                for dst in (A, Bd):
                    nc.sync.dma_start(out=dst[:, 0, :], in_=z[:])
                    nc.sync.dma_start(out=dst[:, NBUF - 2, :], in_=z[:])
                    nc.sync.dma_start(out=dst[:, NBUF - 1, :], in_=z[:])

            # ---------------- conv chain ----------------
            def mm(pp, lhsT, rhs, start, stop, tp=None):
                nc.tensor.matmul(pp, lhsT=lhsT, rhs=rhs, start=start, stop=stop,
                                 tile_position=tp)

            def store_pairs(dst, r0, ot, cout=64):
                nc.sync.dma_start(out=dst[0:cout, r0 + 1:r0 + 1 + RB:2, :],
                                  in_=ot[0:cout, :, :])
                nc.sync.dma_start(out=dst[0:cout, r0 + 2:r0 + 2 + RB:2, :],
                                  in_=ot[64:64 + cout, :, :])

            def conv_l1(src, dst, cpool, opool, ppool):
                # k=12 (ch,dy packed by host), cout=64, col-paired rows
                for r0 in BLOCKS:
                    it = cpool.tile([16, RB, WP], bf16, tag="cin1")
                    nc.sync.dma_start(out=it[0:12, 0:RB, :],
                                      in_=src[0:12, r0 + 1:r0 + 1 + RB, :])
                    ot = opool.tile([128, RB // 2, WP], bf16, tag="cout1")
                    nc.vector.memset(ot[:, :, 0:1], 0.0)
                    nc.vector.memset(ot[:, :, WP - 1:WP], 0.0)
                    for q in range(RB // 2):
                        rr = 2 * q
                        pp = ppool.tile([128, W], f32, tag="pp", name="p1%d" % q)
                        for dx in range(3):
                            lt = wt["wl1"][0:12, dx * 64:(dx + 1) * 64]
                            mm(pp[0:64], lt, it[0:12, rr, dx:dx + W],
                               dx == 0, dx == 2)
                            mm(pp[64:128], lt, it[0:12, rr + 1, dx:dx + W],
                               dx == 0, dx == 2)
                        nc.scalar.activation(ot[:, q, 1:1 + W], pp[:], AF.Relu,
                                             bias=bt["bl1"][:])
                    store_pairs(dst, r0, ot)

            def conv_p2c(src, dst, ka, kb, bkey, cpool, opool, ppool):
                # cin=64 (dy01 packed + dy2), cout=64, col-paired rows
                for r0 in BLOCKS:
                    it = cpool.tile([128, RB + 2, WP], bf16, tag="cin")
                    nc.sync.dma_start(out=it[0:64, 0:RB + 2, :],
                                      in_=src[0:64, r0:r0 + RB + 2, :])
                    nc.sync.dma_start(out=it[64:128, 0:RB + 2, :],
                                      in_=src[0:64, r0 + 1:r0 + RB + 3, :])
                    ot = opool.tile([128, RB // 2, WP], bf16, tag="cout")
                    nc.vector.memset(ot[:, :, 0:1], 0.0)
                    nc.vector.memset(ot[:, :, WP - 1:WP], 0.0)
                    for q in range(RB // 2):
                        rr = 2 * q
                        pp = ppool.tile([128, W], f32, tag="pp", name="p2%d" % q)
                        for dx in range(3):
                            lt = wt[ka][0:128, dx * 64:(dx + 1) * 64]
                            mm(pp[0:64], lt, it[0:128, rr, dx:dx + W],
                               dx == 0, False)
                            mm(pp[64:128], lt, it[0:128, rr + 1, dx:dx + W],
                               dx == 0, False)
                        for dx in range(3):
                            lb = wt[kb][0:128, dx * 64:(dx + 1) * 64]
                            mm(pp[0:64], lb, it[0:128, rr + 2, dx:dx + W],
                               False, dx == 2)
                            mm(pp[64:128], lb, it[0:128, rr + 3, dx:dx + W],
                               False, dx == 2)
                        nc.scalar.activation(ot[:, q, 1:1 + W], pp[:], AF.Relu,
                                             bias=bt[bkey][:])
                    store_pairs(dst, r0, ot)

            def conv_p2f(src, dst, ka, kb, bkey, cpool, opool, ppool):
                # cin=64 (dy01 packed + dy2), cout=128
                for r0 in BLOCKS:
                    it = cpool.tile([128, RB + 2, WP], bf16, tag="cin")
                    nc.sync.dma_start(out=it[0:64, 0:RB + 2, :],
                                      in_=src[0:64, r0:r0 + RB + 2, :])
                    nc.sync.dma_start(out=it[64:128, 0:RB + 2, :],
                                      in_=src[0:64, r0 + 1:r0 + RB + 3, :])
                    ot = opool.tile([128, RB, WP], bf16, tag="coutf")
                    nc.vector.memset(ot[:, :, 0:1], 0.0)
                    nc.vector.memset(ot[:, :, WP - 1:WP], 0.0)
                    for rr in range(RB):
                        pp = ppool.tile([128, W], f32, tag="pp", name="p3%d" % rr)
                        for dx in range(3):
                            mm(pp[:], wt[ka][0:128, dx * 128:(dx + 1) * 128],
                               it[0:128, rr, dx:dx + W], dx == 0, False)
                        for dx in range(3):
                            mm(pp[:], wt[kb][0:128, dx * 128:(dx + 1) * 128],
                               it[0:128, rr + 2, dx:dx + W], False, dx == 2)
                        nc.scalar.activation(ot[:, rr, 1:1 + W], pp[:], AF.Relu,
                                             bias=bt[bkey][:])
                    nc.sync.dma_start(out=dst[0:128, r0 + 1:r0 + 1 + RB, :],
                                      in_=ot[:, :, :])

            def conv_f3(src, dst, wkey, bkey, cpool, opool, ppool):
                # cin=128, cout=128
                for r0 in BLOCKS:
                    it = cpool.tile([128, RB + 2, WP], bf16, tag="cin")
                    nc.sync.dma_start(out=it[0:128, 0:RB + 2, :],
                                      in_=src[0:128, r0:r0 + RB + 2, :])
                    ot = opool.tile([128, RB, WP], bf16, tag="coutf")
                    nc.vector.memset(ot[:, :, 0:1], 0.0)
                    nc.vector.memset(ot[:, :, WP - 1:WP], 0.0)
                    for rr in range(RB):
                        pp = ppool.tile([128, W], f32, tag="pp", name="p4%d" % rr)
                        for dy in range(3):
                            for dx in range(3):
                                g = dy * 3 + dx
                                mm(pp[:], wt[wkey][:, g * 128:(g + 1) * 128],
                                   it[0:128, rr + dy, dx:dx + W],
                                   g == 0, g == 8)
                        nc.scalar.activation(ot[:, rr, 1:1 + W], pp[:], AF.Relu,
                                             bias=bt[bkey][:])
                    nc.sync.dma_start(out=dst[0:128, r0 + 1:r0 + 1 + RB, :],
                                      in_=ot[:, :, :])

            def conv_f3c(src, dst, wkey, bkey, cpool, opool, ppool):
                # cin=128, cout=64, col-paired rows
                for r0 in BLOCKS:
                    it = cpool.tile([128, RB + 2, WP], bf16, tag="cin")
                    nc.sync.dma_start(out=it[0:128, 0:RB + 2, :],
                                      in_=src[0:128, r0:r0 + RB + 2, :])
                    ot = opool.tile([128, RB // 2, WP], bf16, tag="cout")
                    nc.vector.memset(ot[:, :, 0:1], 0.0)
                    nc.vector.memset(ot[:, :, WP - 1:WP], 0.0)
                    for q in range(RB // 2):
                        rr = 2 * q
                        pp = ppool.tile([128, W], f32, tag="pp", name="p5%d" % q)
                        for dy in range(3):
                            for dx in range(3):
                                g = dy * 3 + dx
                                lt = wt[wkey][:, g * 64:(g + 1) * 64]
                                mm(pp[0:64], lt, it[0:128, rr + dy, dx:dx + W],
                                   g == 0, g == 8)
                                mm(pp[64:128], lt,
                                   it[0:128, rr + 1 + dy, dx:dx + W],
                                   g == 0, g == 8)
                        nc.scalar.activation(ot[:, q, 1:1 + W], pp[:], AF.Relu,
                                             bias=bt[bkey][:])
                    store_pairs(dst, r0, ot)

            def conv_p2q(src, dst, ka, kb, bkey, cpool, opool, ppool):
                # cin=64, cout=22, col-quad rows, Exp activation
                for r0 in BLOCKS:
                    it = cpool.tile([128, RB + 2, WP], bf16, tag="cin")
                    nc.sync.dma_start(out=it[0:64, 0:RB + 2, :],
                                      in_=src[0:64, r0:r0 + RB + 2, :])
                    nc.sync.dma_start(out=it[64:128, 0:RB + 2, :],
                                      in_=src[0:64, r0 + 1:r0 + RB + 3, :])
                    ot = opool.tile([128, RB // 4, WP], f32, tag="cout6")
                    for q in range(RB // 4):
                        rr = 4 * q
                        pp = ppool.tile([128, W], f32, tag="pp", name="p6%d" % q)
                        for dx in range(3):
                            lt = wt[ka][0:128, dx * 22:(dx + 1) * 22]
                            mm(pp[0:22], lt, it[0:128, rr, dx:dx + W],
                               dx == 0, False, (0, 0))
                            mm(pp[32:54], lt, it[0:128, rr + 1, dx:dx + W],
                               dx == 0, False, (0, 32))
                            mm(pp[64:86], lt, it[0:128, rr + 2, dx:dx + W],
                               dx == 0, False, (0, 64))
                            mm(pp[96:118], lt, it[0:128, rr + 3, dx:dx + W],
                               dx == 0, False, (0, 96))
                        for dx in range(3):
                            lb = wt[kb][0:128, dx * 22:(dx + 1) * 22]
                            mm(pp[0:22], lb, it[0:128, rr + 2, dx:dx + W],
                               False, dx == 2, (0, 0))
                            mm(pp[32:54], lb, it[0:128, rr + 3, dx:dx + W],
                               False, dx == 2, (0, 32))
                            mm(pp[64:86], lb, it[0:128, rr + 4, dx:dx + W],
                               False, dx == 2, (0, 64))
                            mm(pp[96:118], lb, it[0:128, rr + 5, dx:dx + W],
                               False, dx == 2, (0, 96))
                        nc.scalar.activation(ot[0:118, q, 1:1 + W], pp[0:118],
                                             AF.Exp, bias=bt[bkey][0:118])
                    for g in range(4):
                        nc.sync.dma_start(
                            out=dst[0:22, r0 + 1 + g:r0 + 1 + RB:4, 1:1 + W],
                            in_=ot[32 * g:32 * g + 22, 0:RB // 4, 1:1 + W])

            with (
                tc.tile_pool(name="cin", bufs=3) as cpool,
                tc.tile_pool(name="cout", bufs=3) as opool,
                tc.tile_pool(name="psum", bufs=8, space="PSUM") as ppool,
            ):
                conv_l1(xdy, A, cpool, opool, ppool)
                conv_p2c(A, Bd, "wl2a", "wl2b", "bl2", cpool, opool, ppool)
                conv_p2f(Bd, A, "wl3a", "wl3b", "bl3", cpool, opool, ppool)
                conv_f3(A, Bd, "wl4", "bl4", cpool, opool, ppool)
                conv_f3c(Bd, A, "wl5", "bl5", cpool, opool, ppool)
                conv_p2q(A, C, "wl6a", "wl6b", "bl6", cpool, opool, ppool)

            # ---------------- horizontal pass ----------------
            with tc.tile_pool(name="filt", bufs=2) as fp:
                tts = [(6, 128, 0), (134, 128, 1), (262, 10, 2)]
                for t0, nt, ti in tts:
                    ex = fp.tile([128, K, W], f32, tag="exh")
                    for k in range(K):
                        nc.sync.dma_start(out=ex[0:nt, k, :],
                                          in_=C[k, t0 + 1:t0 + 1 + nt, 1:1 + W])
                    sh = fp.tile([128, W], f32, tag="sh")
                    nc.vector.tensor_add(sh[0:nt], ex[0:nt, 0, :], ex[0:nt, 1, :])
                    for k in range(2, K):
                        nc.vector.tensor_add(sh[0:nt], sh[0:nt], ex[0:nt, k, :])
                    rcp = fp.tile([128, W], f32, tag="rcp")
                    nc.vector.reciprocal(rcp[0:nt], sh[0:nt])
                    for c in range(3):
                        rg = fp.tile([128, W + 10], f32, tag="rg")
                        nc.sync.dma_start(out=rg[0:nt, :],
                                          in_=rgbf[c, t0:t0 + nt, :])
                        acc = fp.tile([128, W], f32, tag="acc")
                        tmp = fp.tile([128, W], f32, tag="tmp")
                        nc.vector.tensor_mul(acc[0:nt], ex[0:nt, 0, :],
                                             rg[0:nt, 0:W])
                        for k in range(1, K):
                            nc.vector.tensor_mul(tmp[0:nt], ex[0:nt, k, :],
                                                 rg[0:nt, k:k + W])
                            nc.vector.tensor_add(acc[0:nt], acc[0:nt], tmp[0:nt])
                        oh = fp.tile([128, W], f32, tag="oh")
                        nc.vector.tensor_mul(oh[0:nt], acc[0:nt], rcp[0:nt])
                        if ti == 0:
                            bc = fp.tile([16, W], f32, tag="bc")
                            for i in range(5):
                                nc.sync.dma_start(out=bc[i:i + 1, :],
                                                  in_=oh[5:6, :])
                            nc.vector.tensor_scalar_mul(
                                oh[0:5], oh[0:5], mk[0:5, 0:1])
                            nc.vector.tensor_scalar_mul(
                                bc[0:5], bc[0:5], mk[0:5, 1:2])
                            nc.vector.tensor_add(oh[0:5], oh[0:5], bc[0:5])
                        if ti == 2:
                            bc = fp.tile([16, W], f32, tag="bc")
                            nc.vector.memset(bc[0:5, :], 0.0)
                            for i in range(5):
                                nc.sync.dma_start(out=bc[5 + i:6 + i, :],
                                                  in_=oh[4:5, :])
                            nc.vector.tensor_scalar_mul(
                                oh[0:10], oh[0:10], mk[0:10, 2:3])
                            nc.vector.tensor_scalar_mul(
                                bc[0:10], bc[0:10], mk[0:10, 3:4])
                            nc.vector.tensor_add(oh[0:10], oh[0:10], bc[0:10])
                        nc.sync.dma_start(out=P[c, t0 - 6:t0 - 6 + nt, :],
                                          in_=oh[0:nt, :])

            # ---------------- vertical pass + blend ----------------
            with tc.tile_pool(name="vert", bufs=2) as vp:
                for r0 in (0, 128):
                    ev = vp.tile([128, K, W], f32, tag="ev")
                    for k in range(K):
                        nc.sync.dma_start(out=ev[:, k, :],
                                          in_=C[K + k, r0 + 12:r0 + 140, 1:1 + W])
                    sv = vp.tile([128, W], f32, tag="sv")
                    nc.vector.tensor_add(sv[:], ev[:, 0, :], ev[:, 1, :])
                    for k in range(2, K):
                        nc.vector.tensor_add(sv[:], sv[:], ev[:, k, :])
                    rcpv = vp.tile([128, W], f32, tag="rcpv")
                    nc.vector.reciprocal(rcpv[:], sv[:])
                    dep = vp.tile([128, W], f32, tag="dep")
                    nc.sync.dma_start(out=dep[:], in_=x[3, r0 + 12:r0 + 140, 1:1 + W])
                    msk = vp.tile([128, W], f32, tag="msk")
                    nc.scalar.activation(msk[:], dep[:], AF.Sigmoid,
                                         bias=cb[:], scale=15.0)
                    for c in range(3):
                        pt = vp.tile([128, K, W], f32, tag="pt")
                        for k in range(K):
                            nc.sync.dma_start(out=pt[:, k, :],
                                              in_=P[c, r0 + k:r0 + k + 128, :])
                        accv = vp.tile([128, W], f32, tag="accv")
                        tmpv = vp.tile([128, W], f32, tag="tmpv")
                        nc.vector.tensor_mul(accv[:], ev[:, 0, :], pt[:, 0, :])
                        for k in range(1, K):
                            nc.vector.tensor_mul(tmpv[:], ev[:, k, :], pt[:, k, :])
                            nc.vector.tensor_add(accv[:], accv[:], tmpv[:])
                        blur = vp.tile([128, W], f32, tag="blur")
                        nc.vector.tensor_mul(blur[:], accv[:], rcpv[:])
                        rgt = vp.tile([128, W], f32, tag="rgt")
                        nc.sync.dma_start(out=rgt[:],
                                          in_=x[c, r0 + 12:r0 + 140, 1:1 + W])
                        dt_ = vp.tile([128, W], f32, tag="dt")
                        nc.vector.tensor_sub(dt_[:], rgt[:], blur[:])
                        nc.vector.tensor_mul(dt_[:], dt_[:], msk[:])
                        ov = vp.tile([128, W], f32, tag="ov")
                        nc.vector.tensor_add(ov[:], blur[:], dt_[:])
                        nc.sync.dma_start(out=out[c, r0:r0 + 128, :], in_=ov[:])

    nc.compile()
    return nc


def _pack_weights(ins):
    import ml_dtypes
    bf = ml_dtypes.bfloat16
    f = np.float32
    w1, w2, w3, w4, w5 = ins["w1"], ins["w2"], ins["w3"], ins["w4"], ins["w5"]
    wcat = np.concatenate([ins["wh"], ins["wv"]], axis=0)  # [22,64,3,3]

    def packl1(w):  # [64,4,3,3] -> [12, 3*64]: [dy*4+ch, dx*64+o]
        o = np.zeros((12, 192), f)
        for dy in range(3):
            for dx in range(3):
                o[dy * 4:(dy + 1) * 4, dx * 64:(dx + 1) * 64] = w[:, :, dy, dx].T
        return o

    def packp2(w):  # [O,64,3,3] -> ([128,3O] dy01, [64,3O] dy2)
        O = w.shape[0]
        a = np.zeros((128, 3 * O), f)
        b = np.zeros((64, 3 * O), f)
        for dx in range(3):
            a[0:64, dx * O:(dx + 1) * O] = w[:, :, 0, dx].T
            a[64:128, dx * O:(dx + 1) * O] = w[:, :, 1, dx].T
            b[:, dx * O:(dx + 1) * O] = w[:, :, 2, dx].T
        return a, b

    def packf3(w):  # [O,128,3,3] -> [128, 9O]
        O = w.shape[0]
        a = np.zeros((128, 9 * O), f)
        for dy in range(3):
            for dx in range(3):
                a[:, (dy * 3 + dx) * O:(dy * 3 + dx + 1) * O] = w[:, :, dy, dx].T
        return a

    def zpad(b):  # [64,3O] -> [128,3O] (upper half zero: dual-layout dy2)
        return np.concatenate([b, np.zeros_like(b)], axis=0)

    w2a, w2b = packp2(w2)
    w3a, w3b = packp2(w3)
    w6a, w6b = packp2(wcat)
    d = {"wl1": packl1(w1), "wl2a": w2a, "wl2b": zpad(w2b),
         "wl3a": w3a, "wl3b": zpad(w3b),
         "wl4": packf3(w4), "wl5": packf3(w5),
         "wl6a": w6a, "wl6b": zpad(w6b)}
    d = {k: np.ascontiguousarray(v.astype(bf)) for k, v in d.items()}

    def bias128(b, dup):
        o = np.zeros((128, 1), f)
        o[0:len(b), 0] = b
        if dup:
            o[64:64 + len(b), 0] = b
        return o

    d["bl1"] = bias128(ins["b1"], True)
    d["bl2"] = bias128(ins["b2"], True)
    d["bl3"] = bias128(ins["b3"], False)
    d["bl4"] = bias128(ins["b4"], False)
    d["bl5"] = bias128(ins["b5"], True)
    b6 = np.concatenate([ins["bh"], ins["bv"]])
    o6 = np.zeros((128, 1), f)
    for g in range(4):
        o6[32 * g:32 * g + 22, 0] = b6
    d["bl6"] = o6
    return d


def _build_in_maps(inputs):
    import ml_dtypes
    bf = ml_dtypes.bfloat16
    rgb = np.asarray(inputs["rgb"], np.float32)
    depth = np.asarray(inputs["depth"], np.float32)
    wd = _pack_weights({k: np.asarray(v, np.float32) for k, v in inputs.items()
                        if k not in ("rgb", "depth")})

    x_full = np.concatenate([rgb, depth], axis=1)  # [B,4,H,W]
    rgb_pad = np.pad(rgb, ((0, 0), (0, 0), (11, 11), (5, 5)), mode="edge")

    in_maps = []
    for core in range(8):
        b, half = core // 2, core % 2
        s = half * RS
        xs = np.zeros((4, NBUF, WP), np.float32)
        lo, hi = s - 11, s + RS + 11
        clo, chi = max(lo, 0), min(hi, H)
        xs[:, 1 + (clo - lo):1 + (chi - lo), 1:1 + W] = x_full[b, :, clo:chi, :]
        # dy-packed bf16 conv input: xdy[dy*4+ch, j, c] = xs[ch, j+dy-1, c]
        xsb = xs.astype(bf)
        xdy = np.zeros((12, NBUF, WP), bf)
        xdy[0:4, 1:NBUF, :] = xsb[:, 0:NBUF - 1, :]
        xdy[4:8, :, :] = xsb
        xdy[8:12, 0:NBUF - 1, :] = xsb[:, 1:NBUF, :]
        rf = np.ascontiguousarray(rgb_pad[b, :, s:s + HS, :])
        keep_top = 0.0 if half == 0 else 1.0
        keep_bot = 0.0 if half == 1 else 1.0
        mrows = np.zeros((4, 5), np.float32)
        mrows[0, :] = keep_top
        mrows[1, :] = 1.0 - keep_top
        mrows[2, :] = keep_bot
        mrows[3, :] = 1.0 - keep_bot
        m = {"x": xs, "xdy": xdy, "rgbf": rf, "masks": mrows}
        m.update(wd)
        in_maps.append(m)
    return in_maps


def _unshard(results):
    outp = np.zeros((B, 3, H, W), np.float32)
    for core in range(8):
        b, half = core // 2, core % 2
        s = half * RS
        outp[b, :, s:s + RS, :] = results[core]
    return outp


_exec_cache = None


def _get_exec():
    """Build the Bass program and a REUSABLE jitted dispatcher once.

    run_bass_via_pjrt builds a fresh jax.jit closure per call, so every
    warm call re-traces and re-lowers through XLA (~1.5s). Build the
    sharded executable a single time instead.
    """
    global _exec_cache
    if _exec_cache is not None:
        return _exec_cache

    import jax
    from jax.sharding import Mesh, PartitionSpec
    from concourse import mybir
    from concourse.bass2jax import (_bass_exec_p, partition_id_tensor,
                                    install_neuronx_cc_hook)

    nc = _build_program()
    install_neuronx_cc_hook()

    partition_name = (nc.partition_id_tensor.name
                      if nc.partition_id_tensor else None)
    in_names, out_names, out_avals, zero_shapes = [], [], [], []
    for alloc in nc.m.functions[0].allocations:
        if not isinstance(alloc, mybir.MemoryLocationSet):
            continue
        name = alloc.memorylocations[0].name
        if alloc.kind == "ExternalInput":
            if name != partition_name:
                in_names.append(name)
        elif alloc.kind == "ExternalOutput":
            shape = tuple(alloc.tensor_shape)
            dtype = mybir.dt.np(alloc.dtype)
            out_names.append(name)
            out_avals.append(jax.core.ShapedArray(shape, dtype))
            zero_shapes.append((shape, dtype))
    n_params = len(in_names)
    n_outs = len(out_avals)
    in_names_all = in_names + out_names
    if partition_name is not None:
        in_names_all.append(partition_name)
    donate = tuple(range(n_params, n_params + n_outs))

    def _body(*args):
        operands = list(args)
        if partition_name is not None:
            operands.append(partition_id_tensor())
        outs = _bass_exec_p.bind(
            *operands,
            out_avals=tuple(out_avals),
            in_names=tuple(in_names_all),
            out_names=tuple(out_names),
            lowering_input_output_aliases=(),
            sim_require_finite=True,
            sim_require_nnan=True,
            nc=nc,
        )
        return tuple(outs)

    devices = jax.devices()[:8]
    mesh = Mesh(np.asarray(devices), ("core",))
    in_specs = (PartitionSpec("core"),) * (n_params + n_outs)
    out_specs = (PartitionSpec("core"),) * len(out_names)
    try:
        from jax import shard_map
        smapped = shard_map(_body, mesh=mesh, in_specs=in_specs,
                            out_specs=out_specs, check_vma=False)
    except (ImportError, TypeError):
        from jax.experimental.shard_map import shard_map as shard_map_old
        smapped = shard_map_old(_body, mesh=mesh, in_specs=in_specs,
                                out_specs=out_specs, check_rep=False)
    sharded = jax.jit(smapped, donate_argnums=donate, keep_unused=True)
    _exec_cache = (nc, sharded, in_names, out_names, out_avals, zero_shapes)
    return _exec_cache


def kernel(**inputs):
    nc, sharded, in_names, out_names, out_avals, zero_shapes = _get_exec()
    in_maps = _build_in_maps(inputs)
    concat_in = [np.concatenate([np.asarray(m[name])[None] for m in in_maps],
                                axis=0).reshape(8 * in_maps[0][name].shape[0],
                                                *in_maps[0][name].shape[1:])
                 for name in in_names]
    concat_zeros = [np.zeros((8 * s[0], *s[1:]), dt) for s, dt in zero_shapes]
    out_arrs = sharded(*concat_in, *concat_zeros)
    oidx = out_names.index("out")
    res = np.asarray(out_arrs[oidx]).reshape(8, *out_avals[oidx].shape)
    return _unshard(res)


_last_exec_ns = None


def kernel_traced(**inputs):
    """Like kernel(), but captures an NTFF profile and sets _last_exec_ns to
    the per-core NEFF execution time (the true HW exec time)."""
    global _prog, _last_exec_ns
    import sys as _sys, types as _types
    if "antenv.axon_hooks" not in _sys.modules:
        hm = _types.ModuleType("antenv.axon_hooks")
        hm._hook = None
        hm.set_axon_ntff_profile_hook = lambda h: setattr(hm, "_hook", h)
        hm.get_axon_ntff_profile_hook = lambda: hm._hook
        _sys.modules["antenv.axon_hooks"] = hm
    from antenv.axon_hooks import set_axon_ntff_profile_hook
    from trn_agent_boot.trn_boot import _ntff_profile_via_ctypes
    set_axon_ntff_profile_hook(_ntff_profile_via_ctypes("/opt/axon/libaxon_pjrt.so"))
    import concourse.bass_utils as bu
    if not getattr(bu, "_upload_patched", False):
        bu.upload_artifacts = lambda tmpdir: "local://" + tmpdir
        bu._upload_patched = True
    if _prog is None:
        _prog = _build_program()
    in_maps = _build_in_maps(inputs)
    res = bu.run_bass_kernel_spmd(_prog, in_maps, core_ids=list(range(8)),
                                  trace=True, trace_cores=[0])
    if res.exec_time_ns is not None:
        _last_exec_ns = res.exec_time_ns
    trace_path = (res.instructions_and_trace[1]
                  if res.instructions_and_trace else None)
    return _unshard([res.results[c]["out"] for c in range(8)]), trace_path


# revision 10
# speedup vs baseline: 1059.1650x; 1.0623x over previous
"""DFN Bokeh model on 8 TRN2 NeuronCores.

Sharding: 8 shards = (batch b, H-half) pairs; each core gets a 278-row slab
(256 out rows + 11-row conv halo each side, zero-padded at image edges by the
host). Conv chain runs as bf16 matmuls with channels on partitions and PSUM
row accumulation, ping-pong DRAM slabs in bf16. Layers with cout<=64 pack two
output rows into the PE array concurrently via column tiling (PSUM partition
halves); the two softmax-logit convs (cout=22) pack four rows via column
quads. The separable per-pixel filter runs with image rows on partitions; the
horizontal-pass output is bounced through a DRAM "plane" with replicate rows
so the vertical taps become plain row-offset DMA loads.
"""

import numpy as np
import sys

sys.path.insert(0, "/opt/trn_rl_repo")

B, H, W = 4, 512, 512
K = 11
RS = 256          # out rows per core
HS = RS + 22      # slab rows (conv halo 11 each side)
NBUF = HS + 3     # slab rows incl. zero conv pad (+1 for dual-half reads)
WP = W + 2        # slab cols incl. zero conv pad
RB = 8            # conv rows per block (PSUM banks)
# all blocks are a full RB rows; the last block overlaps the previous one
# (recomputes 2 rows) so pair/quad packing never sees a ragged block
BLOCKS = list(range(0, HS - RB, RB)) + [HS - RB]

_prog = None


def _build_program():
    import concourse.bacc as bacc
    import concourse.mybir as mybir
    from concourse import tile

    f32 = mybir.dt.float32
    bf16 = mybir.dt.bfloat16
    AF = mybir.ActivationFunctionType

    nc = bacc.Bacc("TRN2", target_bir_lowering=False, debug=False, num_devices=8)

    x = nc.dram_tensor("x", [4, NBUF, WP], f32, kind="ExternalInput").ap()
    xdy = nc.dram_tensor("xdy", [12, NBUF, WP], bf16, kind="ExternalInput").ap()
    rgbf = nc.dram_tensor("rgbf", [3, HS, W + 10], f32, kind="ExternalInput").ap()
    masks = nc.dram_tensor("masks", [4, 5], f32, kind="ExternalInput").ap()
    wnames = {
        "wl1": (128, 192), "wl2a": (128, 192), "wl2b": (128, 192),
        "wl3a": (128, 384), "wl3b": (128, 384), "wl4": (128, 1152),
        "wl5": (128, 576), "w6ea": (128, 66), "w6eb": (128, 66),
        "w6oa": (128, 66), "w6ob": (128, 66),
    }
    wd = {n: nc.dram_tensor(n, list(s), bf16, kind="ExternalInput").ap()
          for n, s in wnames.items()}
    bnames = ["bl1", "bl2", "bl3", "bl4", "bl5", "bl6"]
    bd = {n: nc.dram_tensor(n, [128, 1], f32, kind="ExternalInput").ap()
          for n in bnames}
    out = nc.dram_tensor("out", [3, RS, W], f32, kind="ExternalOutput").ap()

    with tile.TileContext(nc) as tc:
        with (
            tc.tile_pool(name="dram", bufs=1, space="DRAM") as dpool,
            tc.tile_pool(name="wts", bufs=1) as wpool,
        ):
            A = dpool.tile([128, NBUF, WP], bf16, tag="A")
            Bd = dpool.tile([128, NBUF, WP], bf16, tag="B")
            C = dpool.tile([22, NBUF, WP], f32, tag="C")
            P = dpool.tile([3, HS - 12 + 10, W], f32, tag="P")  # 266 rows

            wt = {}
            for n, s in wnames.items():
                t = wpool.tile(list(s), bf16, tag=n)
                nc.sync.dma_start(out=t[:], in_=wd[n][:])
                wt[n] = t
            bt = {}
            for n in bnames:
                t = wpool.tile([128, 1], f32, tag=n)
                nc.sync.dma_start(out=t[:], in_=bd[n][:])
                bt[n] = t
            # mk cols: 0=keep_top 1=rep_top (partitions 0-4),
            #          2=keep_bot 3=rep_bot (partitions 5-9)
            mk = wpool.tile([16, 4], f32, tag="mk")
            nc.vector.memset(mk[0:5, 2:3], 1.0)
            nc.vector.memset(mk[0:5, 3:4], 0.0)
            nc.sync.dma_start(out=mk[0:5, 0:1], in_=masks[0:1, :])
            nc.sync.dma_start(out=mk[0:5, 1:2], in_=masks[1:2, :])
            nc.sync.dma_start(out=mk[5:10, 2:3], in_=masks[2:3, :])
            nc.sync.dma_start(out=mk[5:10, 3:4], in_=masks[3:4, :])

            cb = wpool.tile([128, 1], f32, tag="cb")
            nc.vector.memset(cb[:], -3.0)

            # zero the pad rows of A and B
            with tc.tile_pool(name="zz", bufs=1) as zpool:
                z = zpool.tile([128, WP], bf16, tag="z")
                nc.vector.memset(z[:], 0.0)
                for dst in (A, Bd):
                    nc.sync.dma_start(out=dst[:, 0, :], in_=z[:])
                    nc.sync.dma_start(out=dst[:, NBUF - 2, :], in_=z[:])
                    nc.sync.dma_start(out=dst[:, NBUF - 1, :], in_=z[:])

            # ---------------- conv chain ----------------
            def mm(pp, lhsT, rhs, start, stop, tp=None):
                nc.tensor.matmul(pp, lhsT=lhsT, rhs=rhs, start=start, stop=stop,
                                 tile_position=tp)

            def store_pairs(dst, r0, ot, cout=64):
                nc.sync.dma_start(out=dst[0:cout, r0 + 1:r0 + 1 + RB:2, :],
                                  in_=ot[0:cout, :, :])
                nc.sync.dma_start(out=dst[0:cout, r0 + 2:r0 + 2 + RB:2, :],
                                  in_=ot[64:64 + cout, :, :])

            def conv_l1(src, dst, cpool, opool, ppool):
                # k padded 12->128 with zero weights so the PE array reads as
                # busy and HAM unthrottles; partitions 12-127 of the input
                # tiles are zeroed once up front (pool buffers are stable)
                for i in range(3):
                    it = cpool.tile([128, RB, WP], bf16, tag="cin1")
                    nc.vector.memset(it[:, :, :], 0.0)
                for r0 in BLOCKS:
                    it = cpool.tile([128, RB, WP], bf16, tag="cin1")
                    nc.sync.dma_start(out=it[0:12, 0:RB, :],
                                      in_=src[0:12, r0 + 1:r0 + 1 + RB, :])
                    ot = opool.tile([128, RB // 2, WP], bf16, tag="cout1")
                    nc.vector.memset(ot[:, :, 0:1], 0.0)
                    nc.vector.memset(ot[:, :, WP - 1:WP], 0.0)
                    for q in range(RB // 2):
                        rr = 2 * q
                        pp = ppool.tile([128, W], f32, tag="pp", name="p1%d" % q)
                        for dx in range(3):
                            lt = wt["wl1"][0:128, dx * 64:(dx + 1) * 64]
                            mm(pp[0:64], lt, it[0:128, rr, dx:dx + W],
                               dx == 0, dx == 2)
                            mm(pp[64:128], lt, it[0:128, rr + 1, dx:dx + W],
                               dx == 0, dx == 2)
                        nc.scalar.activation(ot[:, q, 1:1 + W], pp[:], AF.Relu,
                                             bias=bt["bl1"][:])
                    store_pairs(dst, r0, ot)

            def conv_p2c(src, dst, ka, kb, bkey, cpool, opool, ppool):
                # cin=64 (dy01 packed + dy2), cout=64, col-paired rows
                for r0 in BLOCKS:
                    it = cpool.tile([128, RB + 2, WP], bf16, tag="cin")
                    nc.sync.dma_start(out=it[0:64, 0:RB + 2, :],
                                      in_=src[0:64, r0:r0 + RB + 2, :])
                    nc.sync.dma_start(out=it[64:128, 0:RB + 2, :],
                                      in_=src[0:64, r0 + 1:r0 + RB + 3, :])
                    ot = opool.tile([128, RB // 2, WP], bf16, tag="cout")
                    nc.vector.memset(ot[:, :, 0:1], 0.0)
                    nc.vector.memset(ot[:, :, WP - 1:WP], 0.0)
                    for q in range(RB // 2):
                        rr = 2 * q
                        pp = ppool.tile([128, W], f32, tag="pp", name="p2%d" % q)
                        for dx in range(3):
                            lt = wt[ka][0:128, dx * 64:(dx + 1) * 64]
                            mm(pp[0:64], lt, it[0:128, rr, dx:dx + W],
                               dx == 0, False)
                            mm(pp[64:128], lt, it[0:128, rr + 1, dx:dx + W],
                               dx == 0, False)
                        for dx in range(3):
                            lb = wt[kb][0:128, dx * 64:(dx + 1) * 64]
                            mm(pp[0:64], lb, it[0:128, rr + 2, dx:dx + W],
                               False, dx == 2)
                            mm(pp[64:128], lb, it[0:128, rr + 3, dx:dx + W],
                               False, dx == 2)
                        nc.scalar.activation(ot[:, q, 1:1 + W], pp[:], AF.Relu,
                                             bias=bt[bkey][:])
                    store_pairs(dst, r0, ot)

            def conv_p2f(src, dst, ka, kb, bkey, cpool, opool, ppool):
                # cin=64 (dy01 packed + dy2), cout=128
                for r0 in BLOCKS:
                    it = cpool.tile([128, RB + 2, WP], bf16, tag="cin")
                    nc.sync.dma_start(out=it[0:64, 0:RB + 2, :],
                                      in_=src[0:64, r0:r0 + RB + 2, :])
                    nc.sync.dma_start(out=it[64:128, 0:RB + 2, :],
                                      in_=src[0:64, r0 + 1:r0 + RB + 3, :])
                    ot = opool.tile([128, RB, WP], bf16, tag="coutf")
                    nc.vector.memset(ot[:, :, 0:1], 0.0)
                    nc.vector.memset(ot[:, :, WP - 1:WP], 0.0)
                    for rr in range(RB):
                        pp = ppool.tile([128, W], f32, tag="pp", name="p3%d" % rr)
                        for dx in range(3):
                            mm(pp[:], wt[ka][0:128, dx * 128:(dx + 1) * 128],
                               it[0:128, rr, dx:dx + W], dx == 0, False)
                        for dx in range(3):
                            mm(pp[:], wt[kb][0:128, dx * 128:(dx + 1) * 128],
                               it[0:128, rr + 2, dx:dx + W], False, dx == 2)
                        nc.scalar.activation(ot[:, rr, 1:1 + W], pp[:], AF.Relu,
                                             bias=bt[bkey][:])
                    nc.sync.dma_start(out=dst[0:128, r0 + 1:r0 + 1 + RB, :],
                                      in_=ot[:, :, :])

            def conv_f3(src, dst, wkey, bkey, cpool, opool, ppool):
                # cin=128, cout=128
                for r0 in BLOCKS:
                    it = cpool.tile([128, RB + 2, WP], bf16, tag="cin")
                    nc.sync.dma_start(out=it[0:128, 0:RB + 2, :],
                                      in_=src[0:128, r0:r0 + RB + 2, :])
                    ot = opool.tile([128, RB, WP], bf16, tag="coutf")
                    nc.vector.memset(ot[:, :, 0:1], 0.0)
                    nc.vector.memset(ot[:, :, WP - 1:WP], 0.0)
                    for rr in range(RB):
                        pp = ppool.tile([128, W], f32, tag="pp", name="p4%d" % rr)
                        for dy in range(3):
                            for dx in range(3):
                                g = dy * 3 + dx
                                mm(pp[:], wt[wkey][:, g * 128:(g + 1) * 128],
                                   it[0:128, rr + dy, dx:dx + W],
                                   g == 0, g == 8)
                        nc.scalar.activation(ot[:, rr, 1:1 + W], pp[:], AF.Relu,
                                             bias=bt[bkey][:])
                    nc.sync.dma_start(out=dst[0:128, r0 + 1:r0 + 1 + RB, :],
                                      in_=ot[:, :, :])

            def conv_l5l6(src, dst, cpool, opool, ppool, rpool):
                # L5 (cin=128, cout=64, col-paired) writes pairs into an SBUF
                # ring; L6 (cout=22, col-quad, Exp) consumes the ring one
                # block behind. Ring slot sp(p) = p % 12; slot 12 stays zero
                # for the boundary taps (src rows 0 and 279).
                ring = rpool.tile([128, 13, WP], bf16, tag="ring")
                nc.vector.memset(ring[:, 12, :], 0.0)
                nc.vector.memset(ring[:, :, 0:1], 0.0)
                nc.vector.memset(ring[:, :, WP - 1:WP], 0.0)

                def sp(p):
                    return p % 12 if 0 <= p <= 138 else 12

                def l5_block(r0):
                    it = cpool.tile([128, RB + 2, WP], bf16, tag="cin")
                    nc.sync.dma_start(out=it[0:128, 0:RB + 2, :],
                                      in_=src[0:128, r0:r0 + RB + 2, :])
                    for q in range(RB // 2):
                        rr = 2 * q
                        pp = ppool.tile([128, W], f32, tag="pp", name="p5%d" % q)
                        for dy in range(3):
                            for dx in range(3):
                                g = dy * 3 + dx
                                lt = wt["wl5"][:, g * 64:(g + 1) * 64]
                                mm(pp[0:64], lt, it[0:128, rr + dy, dx:dx + W],
                                   g == 0, g == 8)
                                mm(pp[64:128], lt,
                                   it[0:128, rr + 1 + dy, dx:dx + W],
                                   g == 0, g == 8)
                        nc.scalar.activation(
                            ring[:, sp(r0 // 2 + q), 1:1 + W], pp[:], AF.Relu,
                            bias=bt["bl5"][:])

                def l6_block(r0):
                    ot = opool.tile([128, RB // 4, WP], f32, tag="cout6")
                    for q in range(RB // 4):
                        t0 = r0 + 1 + 4 * q
                        pp = ppool.tile([128, W], f32, tag="pp", name="p6%d" % q)
                        for i in range(4):
                            t = t0 + i
                            cg = 32 * i
                            o = pp[cg:cg + 22]
                            if t % 2 == 1:
                                p = (t - 1) // 2
                                grps = [("w6oa", sp(p)), ("w6ob", sp(p - 1))]
                            else:
                                p = (t - 2) // 2
                                grps = [("w6ea", sp(p)), ("w6eb", sp(p + 1))]
                            for gi, (wk, sl) in enumerate(grps):
                                for dx in range(3):
                                    mm(o, wt[wk][0:128, dx * 22:(dx + 1) * 22],
                                       ring[:, sl, dx:dx + W],
                                       gi == 0 and dx == 0,
                                       gi == 1 and dx == 2, (0, cg))
                        nc.scalar.activation(ot[0:118, q, 1:1 + W], pp[0:118],
                                             AF.Exp, bias=bt["bl6"][0:118])
                    for g in range(4):
                        nc.sync.dma_start(
                            out=dst[0:22, r0 + 1 + g:r0 + 1 + RB:4, 1:1 + W],
                            in_=ot[32 * g:32 * g + 22, 0:RB // 4, 1:1 + W])

                for bi, r0 in enumerate(BLOCKS):
                    l5_block(r0)
                    if bi > 0:
                        l6_block(BLOCKS[bi - 1])
                l6_block(BLOCKS[-1])

            with (
                tc.tile_pool(name="cin", bufs=3) as cpool,
                tc.tile_pool(name="cout", bufs=3) as opool,
                tc.tile_pool(name="psum", bufs=8, space="PSUM") as ppool,
            ):
                conv_l1(xdy, A, cpool, opool, ppool)
                conv_p2c(A, Bd, "wl2a", "wl2b", "bl2", cpool, opool, ppool)
                conv_p2f(Bd, A, "wl3a", "wl3b", "bl3", cpool, opool, ppool)
                conv_f3(A, Bd, "wl4", "bl4", cpool, opool, ppool)
                conv_l5l6(Bd, C, cpool, opool, ppool, wpool)

            # ---------------- horizontal pass ----------------
            with tc.tile_pool(name="filt", bufs=2) as fp:
                tts = [(6, 128, 0), (134, 128, 1), (262, 10, 2)]
                for t0, nt, ti in tts:
                    ex = fp.tile([128, K, W], f32, tag="exh")
                    for k in range(K):
                        nc.sync.dma_start(out=ex[0:nt, k, :],
                                          in_=C[k, t0 + 1:t0 + 1 + nt, 1:1 + W])
                    sh = fp.tile([128, W], f32, tag="sh")
                    nc.vector.tensor_add(sh[0:nt], ex[0:nt, 0, :], ex[0:nt, 1, :])
                    for k in range(2, K):
                        nc.vector.tensor_add(sh[0:nt], sh[0:nt], ex[0:nt, k, :])
                    rcp = fp.tile([128, W], f32, tag="rcp")
                    nc.vector.reciprocal(rcp[0:nt], sh[0:nt])
                    for c in range(3):
                        rg = fp.tile([128, W + 10], f32, tag="rg")
                        nc.sync.dma_start(out=rg[0:nt, :],
                                          in_=rgbf[c, t0:t0 + nt, :])
                        acc = fp.tile([128, W], f32, tag="acc")
                        tmp = fp.tile([128, W], f32, tag="tmp")
                        nc.vector.tensor_mul(acc[0:nt], ex[0:nt, 0, :],
                                             rg[0:nt, 0:W])
                        for k in range(1, K):
                            nc.vector.tensor_mul(tmp[0:nt], ex[0:nt, k, :],
                                                 rg[0:nt, k:k + W])
                            nc.vector.tensor_add(acc[0:nt], acc[0:nt], tmp[0:nt])
                        oh = fp.tile([128, W], f32, tag="oh")
                        nc.vector.tensor_mul(oh[0:nt], acc[0:nt], rcp[0:nt])
                        if ti == 0:
                            bc = fp.tile([16, W], f32, tag="bc")
                            for i in range(5):
                                nc.sync.dma_start(out=bc[i:i + 1, :],
                                                  in_=oh[5:6, :])
                            nc.vector.tensor_scalar_mul(
                                oh[0:5], oh[0:5], mk[0:5, 0:1])
                            nc.vector.tensor_scalar_mul(
                                bc[0:5], bc[0:5], mk[0:5, 1:2])
                            nc.vector.tensor_add(oh[0:5], oh[0:5], bc[0:5])
                        if ti == 2:
                            bc = fp.tile([16, W], f32, tag="bc")
                            nc.vector.memset(bc[0:5, :], 0.0)
                            for i in range(5):
                                nc.sync.dma_start(out=bc[5 + i:6 + i, :],
                                                  in_=oh[4:5, :])
                            nc.vector.tensor_scalar_mul(
                                oh[0:10], oh[0:10], mk[0:10, 2:3])
                            nc.vector.tensor_scalar_mul(
                                bc[0:10], bc[0:10], mk[0:10, 3:4])
                            nc.vector.tensor_add(oh[0:10], oh[0:10], bc[0:10])
                        nc.sync.dma_start(out=P[c, t0 - 6:t0 - 6 + nt, :],
                                          in_=oh[0:nt, :])

            # ---------------- vertical pass + blend ----------------
            with tc.tile_pool(name="vert", bufs=2) as vp:
                for r0 in (0, 128):
                    ev = vp.tile([128, K, W], f32, tag="ev")
                    for k in range(K):
                        nc.sync.dma_start(out=ev[:, k, :],
                                          in_=C[K + k, r0 + 12:r0 + 140, 1:1 + W])
                    sv = vp.tile([128, W], f32, tag="sv")
                    nc.vector.tensor_add(sv[:], ev[:, 0, :], ev[:, 1, :])
                    for k in range(2, K):
                        nc.vector.tensor_add(sv[:], sv[:], ev[:, k, :])
                    rcpv = vp.tile([128, W], f32, tag="rcpv")
                    nc.vector.reciprocal(rcpv[:], sv[:])
                    dep = vp.tile([128, W], f32, tag="dep")
                    nc.sync.dma_start(out=dep[:], in_=x[3, r0 + 12:r0 + 140, 1:1 + W])
                    msk = vp.tile([128, W], f32, tag="msk")
                    nc.scalar.activation(msk[:], dep[:], AF.Sigmoid,
                                         bias=cb[:], scale=15.0)
                    for c in range(3):
                        pt = vp.tile([128, K, W], f32, tag="pt")
                        for k in range(K):
                            nc.sync.dma_start(out=pt[:, k, :],
                                              in_=P[c, r0 + k:r0 + k + 128, :])
                        accv = vp.tile([128, W], f32, tag="accv")
                        tmpv = vp.tile([128, W], f32, tag="tmpv")
                        nc.vector.tensor_mul(accv[:], ev[:, 0, :], pt[:, 0, :])
                        for k in range(1, K):
                            nc.vector.tensor_mul(tmpv[:], ev[:, k, :], pt[:, k, :])
                            nc.vector.tensor_add(accv[:], accv[:], tmpv[:])
                        blur = vp.tile([128, W], f32, tag="blur")
                        nc.vector.tensor_mul(blur[:], accv[:], rcpv[:])
                        rgt = vp.tile([128, W], f32, tag="rgt")
                        nc.sync.dma_start(out=rgt[:],
                                          in_=x[c, r0 + 12:r0 + 140, 1:1 + W])
                        dt_ = vp.tile([128, W], f32, tag="dt")
                        nc.vector.tensor_sub(dt_[:], rgt[:], blur[:])
                        nc.vector.tensor_mul(dt_[:], dt_[:], msk[:])
                        ov = vp.tile([128, W], f32, tag="ov")
                        nc.vector.tensor_add(ov[:], blur[:], dt_[:])
                        nc.sync.dma_start(out=out[c, r0:r0 + 128, :], in_=ov[:])

    nc.compile()
    return nc


def _pack_weights(ins):
    import ml_dtypes
    bf = ml_dtypes.bfloat16
    f = np.float32
    w1, w2, w3, w4, w5 = ins["w1"], ins["w2"], ins["w3"], ins["w4"], ins["w5"]
    wcat = np.concatenate([ins["wh"], ins["wv"]], axis=0)  # [22,64,3,3]

    def packl1(w):  # [64,4,3,3] -> [128, 3*64] zero-padded: [dy*4+ch, dx*64+o]
        o = np.zeros((128, 192), f)
        for dy in range(3):
            for dx in range(3):
                o[dy * 4:(dy + 1) * 4, dx * 64:(dx + 1) * 64] = w[:, :, dy, dx].T
        return o

    def packp2(w):  # [O,64,3,3] -> ([128,3O] dy01, [64,3O] dy2)
        O = w.shape[0]
        a = np.zeros((128, 3 * O), f)
        b = np.zeros((64, 3 * O), f)
        for dx in range(3):
            a[0:64, dx * O:(dx + 1) * O] = w[:, :, 0, dx].T
            a[64:128, dx * O:(dx + 1) * O] = w[:, :, 1, dx].T
            b[:, dx * O:(dx + 1) * O] = w[:, :, 2, dx].T
        return a, b

    def packf3(w):  # [O,128,3,3] -> [128, 9O]
        O = w.shape[0]
        a = np.zeros((128, 9 * O), f)
        for dy in range(3):
            for dx in range(3):
                a[:, (dy * 3 + dx) * O:(dy * 3 + dx + 1) * O] = w[:, :, dy, dx].T
        return a

    def zpad(b):  # [64,3O] -> [128,3O] (upper half zero: dual-layout dy2)
        return np.concatenate([b, np.zeros_like(b)], axis=0)

    w2a, w2b = packp2(w2)
    w3a, w3b = packp2(w3)

    def packl6(dylo, dyhi):  # [128, 3*22]: lower half tap dylo, upper dyhi
        o = np.zeros((128, 66), f)
        for dx in range(3):
            if dylo is not None:
                o[0:64, dx * 22:(dx + 1) * 22] = wcat[:, :, dylo, dx].T
            if dyhi is not None:
                o[64:128, dx * 22:(dx + 1) * 22] = wcat[:, :, dyhi, dx].T
        return o

    d = {"wl1": packl1(w1), "wl2a": w2a, "wl2b": zpad(w2b),
         "wl3a": w3a, "wl3b": zpad(w3b),
         "wl4": packf3(w4), "wl5": packf3(w5),
         "w6ea": packl6(0, 1), "w6eb": packl6(2, None),
         "w6oa": packl6(1, 2), "w6ob": packl6(None, 0)}
    d = {k: np.ascontiguousarray(v.astype(bf)) for k, v in d.items()}

    def bias128(b, dup):
        o = np.zeros((128, 1), f)
        o[0:len(b), 0] = b
        if dup:
            o[64:64 + len(b), 0] = b
        return o

    d["bl1"] = bias128(ins["b1"], True)
    d["bl2"] = bias128(ins["b2"], True)
    d["bl3"] = bias128(ins["b3"], False)
    d["bl4"] = bias128(ins["b4"], False)
    d["bl5"] = bias128(ins["b5"], True)
    b6 = np.concatenate([ins["bh"], ins["bv"]])
    o6 = np.zeros((128, 1), f)
    for g in range(4):
        o6[32 * g:32 * g + 22, 0] = b6
    d["bl6"] = o6
    return d


def _build_in_maps(inputs):
    import ml_dtypes
    bf = ml_dtypes.bfloat16
    rgb = np.asarray(inputs["rgb"], np.float32)
    depth = np.asarray(inputs["depth"], np.float32)
    wd = _pack_weights({k: np.asarray(v, np.float32) for k, v in inputs.items()
                        if k not in ("rgb", "depth")})

    x_full = np.concatenate([rgb, depth], axis=1)  # [B,4,H,W]
    rgb_pad = np.pad(rgb, ((0, 0), (0, 0), (11, 11), (5, 5)), mode="edge")

    in_maps = []
    for core in range(8):
        b, half = core // 2, core % 2
        s = half * RS
        xs = np.zeros((4, NBUF, WP), np.float32)
        lo, hi = s - 11, s + RS + 11
        clo, chi = max(lo, 0), min(hi, H)
        xs[:, 1 + (clo - lo):1 + (chi - lo), 1:1 + W] = x_full[b, :, clo:chi, :]
        # dy-packed bf16 conv input: xdy[dy*4+ch, j, c] = xs[ch, j+dy-1, c]
        xsb = xs.astype(bf)
        xdy = np.zeros((12, NBUF, WP), bf)
        xdy[0:4, 1:NBUF, :] = xsb[:, 0:NBUF - 1, :]
        xdy[4:8, :, :] = xsb
        xdy[8:12, 0:NBUF - 1, :] = xsb[:, 1:NBUF, :]
        rf = np.ascontiguousarray(rgb_pad[b, :, s:s + HS, :])
        keep_top = 0.0 if half == 0 else 1.0
        keep_bot = 0.0 if half == 1 else 1.0
        mrows = np.zeros((4, 5), np.float32)
        mrows[0, :] = keep_top
        mrows[1, :] = 1.0 - keep_top
        mrows[2, :] = keep_bot
        mrows[3, :] = 1.0 - keep_bot
        m = {"x": xs, "xdy": xdy, "rgbf": rf, "masks": mrows}
        m.update(wd)
        in_maps.append(m)
    return in_maps


def _unshard(results):
    outp = np.zeros((B, 3, H, W), np.float32)
    for core in range(8):
        b, half = core // 2, core % 2
        s = half * RS
        outp[b, :, s:s + RS, :] = results[core]
    return outp


_exec_cache = None


def _get_exec():
    """Build the Bass program and a REUSABLE jitted dispatcher once.

    run_bass_via_pjrt builds a fresh jax.jit closure per call, so every
    warm call re-traces and re-lowers through XLA (~1.5s). Build the
    sharded executable a single time instead.
    """
    global _exec_cache
    if _exec_cache is not None:
        return _exec_cache

    import jax
    from jax.sharding import Mesh, PartitionSpec
    from concourse import mybir
    from concourse.bass2jax import (_bass_exec_p, partition_id_tensor,
                                    install_neuronx_cc_hook)

    nc = _build_program()
    install_neuronx_cc_hook()

    partition_name = (nc.partition_id_tensor.name
                      if nc.partition_id_tensor else None)
    in_names, out_names, out_avals, zero_shapes = [], [], [], []
    for alloc in nc.m.functions[0].allocations:
        if not isinstance(alloc, mybir.MemoryLocationSet):
            continue
        name = alloc.memorylocations[0].name
        if alloc.kind == "ExternalInput":
            if name != partition_name:
                in_names.append(name)
        elif alloc.kind == "ExternalOutput":
            shape = tuple(alloc.tensor_shape)
            dtype = mybir.dt.np(alloc.dtype)
            out_names.append(name)
            out_avals.append(jax.core.ShapedArray(shape, dtype))
            zero_shapes.append((shape, dtype))
    n_params = len(in_names)
    n_outs = len(out_avals)
    in_names_all = in_names + out_names
    if partition_name is not None:
        in_names_all.append(partition_name)
    donate = tuple(range(n_params, n_params + n_outs))

    def _body(*args):
        operands = list(args)
        if partition_name is not None:
            operands.append(partition_id_tensor())
        outs = _bass_exec_p.bind(
            *operands,
            out_avals=tuple(out_avals),
            in_names=tuple(in_names_all),
            out_names=tuple(out_names),
            lowering_input_output_aliases=(),
            sim_require_finite=True,
            sim_require_nnan=True,
            nc=nc,
        )
        return tuple(outs)

    devices = jax.devices()[:8]
    mesh = Mesh(np.asarray(devices), ("core",))
    in_specs = (PartitionSpec("core"),) * (n_params + n_outs)
    out_specs = (PartitionSpec("core"),) * len(out_names)
    try:
        from jax import shard_map
        smapped = shard_map(_body, mesh=mesh, in_specs=in_specs,
                            out_specs=out_specs, check_vma=False)
    except (ImportError, TypeError):
        from jax.experimental.shard_map import shard_map as shard_map_old
        smapped = shard_map_old(_body, mesh=mesh, in_specs=in_specs,
                                out_specs=out_specs, check_rep=False)
    sharded = jax.jit(smapped, donate_argnums=donate, keep_unused=True)
    _exec_cache = (nc, sharded, in_names, out_names, out_avals, zero_shapes)
    return _exec_cache


def kernel(**inputs):
    nc, sharded, in_names, out_names, out_avals, zero_shapes = _get_exec()
    in_maps = _build_in_maps(inputs)
    concat_in = [np.concatenate([np.asarray(m[name])[None] for m in in_maps],
                                axis=0).reshape(8 * in_maps[0][name].shape[0],
                                                *in_maps[0][name].shape[1:])
                 for name in in_names]
    concat_zeros = [np.zeros((8 * s[0], *s[1:]), dt) for s, dt in zero_shapes]
    out_arrs = sharded(*concat_in, *concat_zeros)
    oidx = out_names.index("out")
    res = np.asarray(out_arrs[oidx]).reshape(8, *out_avals[oidx].shape)
    return _unshard(res)


_last_exec_ns = None


def kernel_traced(**inputs):
    """Like kernel(), but captures an NTFF profile and sets _last_exec_ns to
    the per-core NEFF execution time (the true HW exec time)."""
    global _prog, _last_exec_ns
    import sys as _sys, types as _types
    if "antenv.axon_hooks" not in _sys.modules:
        hm = _types.ModuleType("antenv.axon_hooks")
        hm._hook = None
        hm.set_axon_ntff_profile_hook = lambda h: setattr(hm, "_hook", h)
        hm.get_axon_ntff_profile_hook = lambda: hm._hook
        _sys.modules["antenv.axon_hooks"] = hm
    from antenv.axon_hooks import set_axon_ntff_profile_hook
    from trn_agent_boot.trn_boot import _ntff_profile_via_ctypes
    set_axon_ntff_profile_hook(_ntff_profile_via_ctypes("/opt/axon/libaxon_pjrt.so"))
    import concourse.bass_utils as bu
    if not getattr(bu, "_upload_patched", False):
        bu.upload_artifacts = lambda tmpdir: "local://" + tmpdir
        bu._upload_patched = True
    if _prog is None:
        _prog = _build_program()
    in_maps = _build_in_maps(inputs)
    res = bu.run_bass_kernel_spmd(_prog, in_maps, core_ids=list(range(8)),
                                  trace=True, trace_cores=[0])
    if res.exec_time_ns is not None:
        _last_exec_ns = res.exec_time_ns
    trace_path = (res.instructions_and_trace[1]
                  if res.instructions_and_trace else None)
    return _unshard([res.results[c]["out"] for c in range(8)]), trace_path
